# revision 1
# baseline (speedup 1.0000x reference)
"""Causal MQA self-attention (RoPE + RMS-norm on q/k) on 8 TRN2 NeuronCores.

Sharding: core c -> (batch b = c//4, head-group g = c%4 of 4 heads).
Each core computes, for its batch and its 4 heads:
  q/k/v projections -> RoPE -> RMS-norm -> causal attention -> partial
  output projection out_part = attn_out_g @ wo[:, g].T  (shape [S, HID]).
Host sums the 4 per-group partials of each batch (row-parallel matmul
unshard) and stacks the 2 batches.

PE-facing tensors are bf16 (fp32 PSUM accumulation); softmax runs
without max-subtraction (post-RMS-norm scores <= sqrt(D) ~ 11.3, exp
in range). Attention output is produced TRANSPOSED ([d, q] = v.T @ p.T
via 512-wide moving matmuls) so it feeds the output projection without
extra transposes; the softmax denominator comes from a [1,512] ones-row
matmul accumulated in PSUM, inverted and partition-broadcast on GpSimd.
"""

import ml_dtypes
import numpy as np

import concourse.bass as bass
import concourse.mybir as mybir
import concourse.tile as tile
from concourse import bacc
from concourse.bass_utils import run_bass_kernel_spmd
from concourse.masks import make_identity

# problem dims (hardcoded per contract)
B, S, HID, H, D = 2, 2048, 2048, 16, 128
NCORES = 8
GROUPS = 4              # head-groups = cores per batch
HG = H // GROUPS        # heads per core
DG = HG * D             # 512 projected q dims per core
NT = S // 128           # 16 sequence tiles
HT = HID // 128         # 16 hidden tiles
NQC = 4                 # q chunks of 512 columns
EPS = 1.1920928955078125e-07
ISD = 1.0 / float(np.sqrt(D))

f32 = mybir.dt.float32
bf16 = mybir.dt.bfloat16

TRACE = False           # test harness may flip this for NTFF profiling
LAST = {}               # last BassKernelResults, for the test harness
PH1_TILES = NT          # bisect knob
PH2_CHUNKS = NQC        # bisect knob

_compiled = None


def _emit(nc, xT, wqT, wkvT, woT, csx, snx, cmw, out):
    add = mybir.AluOpType.add
    Sqrt = mybir.ActivationFunctionType.Sqrt
    Exp = mybir.ActivationFunctionType.Exp

    with tile.TileContext(nc) as tc:
        with (
            tc.tile_pool(name="consts", bufs=1) as consts,
            tc.tile_pool(name="bigp", bufs=1) as bigp,
            tc.tile_pool(name="xsp", bufs=3) as xsp,
            tc.tile_pool(name="csp", bufs=2) as csp,
            tc.tile_pool(name="rsp", bufs=3) as rsp,
            tc.tile_pool(name="smp", bufs=4) as smp,
            tc.tile_pool(name="qnp", bufs=4) as qnp,
            tc.tile_pool(name="ptp", bufs=10) as ptp,
            tc.tile_pool(name="otp", bufs=2) as otp,
            tc.tile_pool(name="ocp", bufs=4) as ocp,
            tc.tile_pool(name="pA", bufs=3, space="PSUM") as pA,
            tc.tile_pool(name="pX", bufs=2, space="PSUM") as pX,
        ):
            # ---- constants ----
            ident = consts.tile([128, 128], bf16)
            make_identity(nc, ident)
            eps_t = consts.tile([128, 1], f32)
            nc.vector.memset(eps_t, EPS)
            cmw_sb = consts.tile([128, NQC, DG], bf16)  # wide causal masks
            nc.sync.dma_start(cmw_sb, cmw.rearrange("p (k q) -> p k q", k=NQC))

            # ---- resident weights / activations ----
            wq_sb = bigp.tile([128, HT, DG], bf16, tag="wq")
            nc.sync.dma_start(wq_sb, wqT.rearrange("(t p) d -> p t d", p=128))
            wkv_sb = bigp.tile([128, HT, 2 * D], bf16, tag="wkv")
            nc.sync.dma_start(wkv_sb, wkvT.rearrange("(t p) d -> p t d", p=128))
            wo_sb = bigp.tile([128, HG, HID], bf16, tag="wo")
            nc.sync.dma_start(wo_sb, woT.rearrange("(h p) n -> p h n", p=128))

            qT_all = bigp.tile([128, HG, S], bf16, tag="qT")   # [d, h, s]
            kT_sb = bigp.tile([128, S], bf16, tag="kT")        # [d, s]
            vvb = bigp.tile([128, NT, 132], bf16, tag="vv")    # [s%128, s//128, d|ones]
            nc.vector.memset(vvb[:, :, 128:132], 1.0)

            xTr = xT.rearrange("(t p) s -> p t s", p=128)

            def bcast4(src2d, st):
                base = src2d[st * 128:(st + 1) * 128, :]
                return bass.AP(
                    tensor=base.tensor,
                    offset=base.offset,
                    ap=[base.ap[0], [0, HG], base.ap[1]],
                )

            def emit_st(st):
                xs0 = xsp.tile([128, HT // 2, 128], bf16, tag="xs")
                nc.sync.dma_start(xs0, xTr[:, 0:HT // 2, st * 128:(st + 1) * 128])
                xs1 = xsp.tile([128, HT // 2, 128], bf16, tag="xs")
                nc.sync.dma_start(xs1, xTr[:, HT // 2:HT, st * 128:(st + 1) * 128])
                xhalves = (xs0, xs1)

                cs_t = csp.tile([128, HG, 128], f32, tag="cs")
                nc.gpsimd.dma_start(cs_t, bcast4(csx, st))
                sn_t = csp.tile([128, HG, 128], f32, tag="sn")
                nc.gpsimd.dma_start(sn_t, bcast4(snx, st))

                qp = pA.tile([128, 2, DG], f32, tag="A")
                for t in range(HT):
                    nc.tensor.matmul(
                        qp[:, 0, :], lhsT=xhalves[t // 8][:, t % 8, :],
                        rhs=wq_sb[:, t, :], start=(t == 0), stop=(t == HT - 1),
                    )
                kvp = pA.tile([128, 2, DG], f32, tag="A")
                for t in range(HT):
                    nc.tensor.matmul(
                        kvp[:, 0, 0:2 * D], lhsT=xhalves[t // 8][:, t % 8, :],
                        rhs=wkv_sb[:, t, :], start=(t == 0), stop=(t == HT - 1),
                    )

                # stage kv psum -> SBUF on ACT, then v -> bf16 tiles
                kvs = rsp.tile([128, 2 * D], f32, tag="kvs")
                nc.scalar.copy(kvs, kvp[:, 0, 0:2 * D])
                nc.vector.tensor_copy(vvb[:, st, 0:128], kvs[:, D:2 * D])

                # ---- RoPE + RMS-norm for 4 q heads, batched ----
                # stage psum -> SBUF on ACT (frees the PSUM slot early and
                # lets the DVE ops run in SBUF-only 2x mode)
                qs = rsp.tile([128, DG], f32, tag="qs")
                nc.scalar.copy(qs, qp[:, 0, :])
                q3 = qs.rearrange("p (h d) -> p h d", h=HG)
                q4 = qs.rearrange("p (h t d) -> p h t d", h=HG, t=2)
                rot = rsp.tile([128, DG], f32, tag="rot")
                r4 = rot.rearrange("p (h t d) -> p h t d", h=HG, t=2)
                r3 = rot.rearrange("p (h d) -> p h d", h=HG)
                nc.scalar.copy(r4[:, :, 0, :], q4[:, :, 1, :])
                nc.scalar.mul(r4[:, :, 1, :], q4[:, :, 0, :], -1.0)
                t1 = rsp.tile([128, DG], f32, tag="t1")
                t3 = t1.rearrange("p (h d) -> p h d", h=HG)
                nc.vector.tensor_mul(t3, q3, cs_t)
                nc.vector.tensor_mul(r3, r3, sn_t)
                nc.vector.tensor_add(t1, t1, rot)          # t1 = roped q
                ms4 = smp.tile([128, HG], f32, tag="ms4")
                nc.vector.tensor_mul(rot, t1, t1)          # rot dead; reuse as q^2
                nc.vector.tensor_reduce(
                    ms4, rot.rearrange("p (h d) -> p h d", h=HG),
                    axis=mybir.AxisListType.X, op=add)
                srt4 = smp.tile([128, HG], f32, tag="srt4")
                nc.scalar.activation(out=srt4, in_=ms4, func=Sqrt,
                                     bias=eps_t[:, 0:1], scale=1.0 / D)
                nc.vector.reciprocal(srt4, srt4)
                for h in range(HG):
                    qn = qnp.tile([128, 128], bf16, tag="qn")
                    nc.vector.tensor_scalar_mul(
                        qn, t1[:, h * 128:(h + 1) * 128], srt4[:, h:h + 1])
                    tp = pX.tile([128, DG], f32, tag="X")
                    nc.tensor.transpose(tp[:, 0:64].bitcast(bf16), qn, ident)
                    nc.scalar.copy(
                        qT_all[:, h, st * 128:(st + 1) * 128],
                        tp[:, 0:64].bitcast(bf16))

                # ---- RoPE + RMS-norm for k (single kv head) ----
                kk = kvs[:, 0:D]
                k2 = kk.rearrange("p (t d) -> p t d", t=2)
                krot = rsp.tile([128, 128], f32, tag="krot")
                kr2 = krot.rearrange("p (t d) -> p t d", t=2)
                nc.scalar.copy(kr2[:, 0, :], k2[:, 1, :])
                nc.scalar.mul(kr2[:, 1, :], k2[:, 0, :], -1.0)
                kt1 = rsp.tile([128, 128], f32, tag="kt1")
                nc.vector.tensor_mul(kt1, kk, cs_t[:, 0, :])
                nc.vector.tensor_mul(krot, krot, sn_t[:, 0, :])
                nc.vector.tensor_add(kt1, kt1, krot)
                msk = smp.tile([128, 1], f32, tag="msk")
                nc.vector.tensor_mul(krot, kt1, kt1)       # krot dead; reuse as k^2
                nc.vector.tensor_reduce(msk, krot, axis=mybir.AxisListType.X, op=add)
                srtk = smp.tile([128, 1], f32, tag="srtk")
                nc.scalar.activation(out=srtk, in_=msk, func=Sqrt,
                                     bias=eps_t[:, 0:1], scale=1.0 / D)
                nc.vector.reciprocal(srtk, srtk)
                kn = qnp.tile([128, 128], bf16, tag="kn")
                nc.vector.tensor_scalar_mul(kn, kt1, srtk)
                tp = pX.tile([128, DG], f32, tag="X")
                nc.tensor.transpose(tp[:, 0:64].bitcast(bf16), kn, ident)
                nc.scalar.copy(
                    kT_sb[:, st * 128:(st + 1) * 128],
                    tp[:, 0:64].bitcast(bf16))

            def emit_qc(qc):
                otile = otp.tile([128, HG, DG], bf16, tag="ot")  # [d, h, q]
                nkt = 4 * (qc + 1)
                for h in range(HG):
                    qrhs = qT_all[:, h, qc * DG:(qc + 1) * DG]
                    pts = []
                    for j2 in range(0, nkt, 2):
                        sp = pA.tile([128, 2, DG], f32, tag="A")
                        for j in range(2):
                            kt = j2 + j
                            nc.tensor.matmul(
                                sp[:, j, :],
                                lhsT=kT_sb[:, kt * 128:(kt + 1) * 128],
                                rhs=qrhs, start=True, stop=True)
                        pt = ptp.tile([128, 2, DG], bf16, tag="pt")
                        # ragged exp: skip fully-masked (k > q) spans of
                        # diagonal-range k tiles; those pT regions are never
                        # read by the causal PV loop below.
                        for j in range(2):
                            kt = j2 + j
                            qoff = max(0, (kt - 4 * qc)) * 128
                            if qoff >= DG:
                                continue
                            nc.scalar.activation(
                                out=pt[:, j, qoff:DG], in_=sp[:, j, qoff:DG],
                                func=Exp, scale=ISD)
                        pts.append(pt)
                    # causal masking of the 4 diagonal k tiles (tri block)
                    for qtl in range(4):
                        kt = 4 * qc + qtl
                        sl = pts[kt // 2][:, kt % 2, qtl * 128:(qtl + 1) * 128]
                        nc.vector.tensor_mul(sl, sl, cmw_sb[:, qtl, qtl * 128:(qtl + 1) * 128])
                    # probs @ [v | ones] per q tile
                    for qtl in range(4):
                        qt = 4 * qc + qtl
                        op = pX.tile([128, DG], f32, tag="X")
                        for kt in range(qt + 1):
                            nc.tensor.matmul(
                                op[:, 0:129],
                                lhsT=pts[kt // 2][:, kt % 2, qtl * 128:(qtl + 1) * 128],
                                rhs=vvb[:, kt, 0:129],
                                start=(kt == 0), stop=(kt == qt))
                        rc = smp.tile([128, 1], f32, tag="rc")
                        nc.vector.reciprocal(rc, op[:, 128:129])
                        on = qnp.tile([128, 128], bf16, tag="on")
                        nc.vector.tensor_scalar_mul(on, op[:, 0:128], rc)
                        tp = pX.tile([128, DG], f32, tag="X")
                        nc.tensor.transpose(tp[:, 0:64].bitcast(bf16), on, ident)
                        nc.vector.tensor_copy(
                            otile[:, h, qtl * 128:(qtl + 1) * 128],
                            tp[:, 0:64].bitcast(bf16))
                # output projection for this chunk's 4 row tiles
                for stl in range(4):
                    srow = (4 * qc + stl) * 128
                    for cc in range(4):
                        wop = pA.tile([128, 2, DG], f32, tag="A")
                        for h2 in range(HG):
                            nc.tensor.matmul(
                                wop[:, 0, :],
                                lhsT=otile[:, h2, stl * 128:(stl + 1) * 128],
                                rhs=wo_sb[:, h2, cc * DG:(cc + 1) * DG],
                                start=(h2 == 0), stop=(h2 == HG - 1))
                        oc = ocp.tile([128, DG], f32, tag="oc")
                        if cc % 2 == 0:
                            nc.vector.tensor_copy(oc, wop[:, 0, :])
                        else:
                            nc.scalar.copy(oc, wop[:, 0, :])
                        nc.sync.dma_start(
                            out[srow:srow + 128, cc * DG:(cc + 1) * DG], oc)



            # interleave: chunk qc only needs s-tiles <= 4*qc+3, so emit its
            # attention right after those tiles -- exp overlaps later proj work
            for st in range(PH1_TILES):
                emit_st(st)
                if st % 4 == 3 and (st // 4) < PH2_CHUNKS:
                    emit_qc(st // 4)


def _build():
    nc = bacc.Bacc("TRN2", target_bir_lowering=False, debug=False,
                   num_devices=NCORES)
    xT = nc.dram_tensor("xT", [HID, S], bf16, kind="ExternalInput").ap()
    wqT = nc.dram_tensor("wqT", [HID, DG], bf16, kind="ExternalInput").ap()
    wkvT = nc.dram_tensor("wkvT", [HID, 2 * D], bf16, kind="ExternalInput").ap()
    woT = nc.dram_tensor("woT", [DG, HID], bf16, kind="ExternalInput").ap()
    csx = nc.dram_tensor("csx", [S, 128], f32, kind="ExternalInput").ap()
    snx = nc.dram_tensor("snx", [S, 128], f32, kind="ExternalInput").ap()
    cmw = nc.dram_tensor("cmw", [128, NQC * DG], bf16, kind="ExternalInput").ap()
    out = nc.dram_tensor("out", [S, HID], f32, kind="ExternalOutput").ap()
    _emit(nc, xT, wqT, wkvT, woT, csx, snx, cmw, out)
    nc.compile()
    return nc


def _get_compiled():
    global _compiled
    if _compiled is None:
        _compiled = _build()
    return _compiled


def _causal_masks():
    """cmw[k, ktl, q]: per diagonal-position wide mask over a 512-q chunk."""
    m = np.zeros((128, NQC, DG), np.float32)
    tri = np.triu(np.ones((128, 128), np.float32))  # 1 where k <= q
    for ktl in range(4):
        for qt in range(4):
            blk = m[:, ktl, qt * 128:(qt + 1) * 128]
            if qt > ktl:
                blk[:] = 1.0
            elif qt == ktl:
                blk[:] = tri
    return np.ascontiguousarray(
        m.reshape(128, NQC * DG).astype(ml_dtypes.bfloat16))


def kernel(x, cos, sin, wq, wk, wv, wo):
    nc = _get_compiled()
    x = np.asarray(x, np.float32)
    cos = np.asarray(cos, np.float32)
    sin = np.asarray(sin, np.float32)
    wq = np.asarray(wq, np.float32)
    wk = np.asarray(wk, np.float32)
    wv = np.asarray(wv, np.float32)
    wo = np.asarray(wo, np.float32)

    bf = ml_dtypes.bfloat16
    wkvT = np.ascontiguousarray(np.concatenate([wk, wv], 0).T.astype(bf))
    csx = np.ascontiguousarray(np.concatenate([cos, cos], 1))
    snx = np.ascontiguousarray(np.concatenate([sin, sin], 1))
    cmw = _causal_masks()
    xTs = [np.ascontiguousarray(x[b].T.astype(bf)) for b in range(B)]
    wqTs = [np.ascontiguousarray(wq[g * DG:(g + 1) * DG].T.astype(bf))
            for g in range(GROUPS)]
    woTs = [np.ascontiguousarray(wo[:, g * DG:(g + 1) * DG].T.astype(bf))
            for g in range(GROUPS)]

    in_maps = []
    for c in range(NCORES):
        b, g = divmod(c, GROUPS)
        in_maps.append({
            "xT": xTs[b], "wqT": wqTs[g], "wkvT": wkvT, "woT": woTs[g],
            "csx": csx, "snx": snx, "cmw": cmw,
        })
    res = run_bass_kernel_spmd(nc, in_maps, list(range(NCORES)), trace=TRACE)
    LAST["res"] = res
    outs = [r["out"] for r in res.results]
    final = np.empty((B, S, HID), np.float32)
    for b in range(B):
        final[b] = (outs[GROUPS * b] + outs[GROUPS * b + 1]
                    + outs[GROUPS * b + 2] + outs[GROUPS * b + 3])
    return final



# revision 4
# speedup vs baseline: 1.0379x; 1.0379x over previous
"""Causal MQA self-attention (RoPE + RMS-norm on q/k) on 8 TRN2 NeuronCores.

Sharding: core c -> (batch b = c//4, head-group g = c%4 of 4 heads).
Each core computes, for its batch and its 4 heads:
  q/k/v projections -> RoPE -> RMS-norm -> causal attention -> partial
  output projection out_part = attn_out_g @ wo[:, g].T  (shape [S, HID]).
Host sums the 4 per-group partials of each batch (row-parallel matmul
unshard) and stacks the 2 batches.

v2 scheduling improvements over the baseline:
  - weight DMAs split into chunks on the scalar-engine HWDGE queue;
    x / cos / sin tile loads on the sync queue -> first matmul at ~2us
    instead of ~34us.
  - x host-repacked into [p, st, t, s] blocks so each x-tile DMA is
    128 x 2KB contiguous descriptors instead of 256B gathers.
  - cos/sin head-broadcast via step-0 free-dim APs on DVE ops instead
    of 4x-replicated gpsimd DMAs.
  - attention chunk emission split into head-pairs (A1/A2) and a
    deferred back-half B (transposes + out-proj + store), interleaved
    with the next stiles so the PE queue always holds solid work while
    the scalar engine drains the exp backlog.
  - PSUM re-layout: 4 single-bank score tiles (deeper exp pipeline),
    shared 2-buf bank tag for PV-accum / transposes / out-proj.
  - diagonal score matmuls narrowed to the unmasked q-span.
  - output stored as bf16 wide rows ([128, 2048], one DMA per row-tile,
    early chunks on the gpsimd queue); partials summed in fp32 on host.
"""

import ml_dtypes
import numpy as np

import concourse.bass as bass
import concourse.mybir as mybir
import concourse.tile as tile
from concourse import bacc
from concourse.bass_utils import run_bass_kernel_spmd
from concourse.masks import make_identity

# problem dims (hardcoded per contract)
B, S, HID, H, D = 2, 2048, 2048, 16, 128
NCORES = 8
GROUPS = 4              # head-groups = cores per batch
HG = H // GROUPS        # heads per core
DG = HG * D             # 512 projected q dims per core
NT = S // 128           # 16 sequence tiles
HT = HID // 128         # 16 hidden tiles
NQC = 4                 # q chunks of 512 columns
EPS = 1.1920928955078125e-07
ISD = 1.0 / float(np.sqrt(D))

f32 = mybir.dt.float32
bf16 = mybir.dt.bfloat16

TRACE = False           # test harness may flip this for NTFF profiling
LAST = {}               # last BassKernelResults, for the test harness
PH1_TILES = NT          # bisect knob
PH2_CHUNKS = NQC        # bisect knob

_compiled = None


def _bc_free(src2d, n):
    """[128, F] tile -> [128, n, F] AP broadcasting along a step-0 mid dim."""
    return bass.AP(
        tensor=src2d.tensor,
        offset=src2d.offset,
        ap=[src2d.ap[0], [0, n], src2d.ap[1]],
    )


def _emit(nc, xT, wqT, wkvT, woT, csx, snx, cmw, out):
    add = mybir.AluOpType.add
    mult = mybir.AluOpType.mult
    Sqrt = mybir.ActivationFunctionType.Sqrt
    Exp = mybir.ActivationFunctionType.Exp

    with tile.TileContext(nc) as tc:
        with (
            tc.tile_pool(name="consts", bufs=1) as consts,
            tc.tile_pool(name="bigp", bufs=1) as bigp,
            tc.tile_pool(name="xsp", bufs=6) as xsp,
            tc.tile_pool(name="csp", bufs=3) as csp,
            tc.tile_pool(name="rsp", bufs=2) as rsp,
            tc.tile_pool(name="smp", bufs=4) as smp,
            tc.tile_pool(name="qnp", bufs=4) as qnp,
            tc.tile_pool(name="ptp", bufs=34) as ptp,
            tc.tile_pool(name="otp", bufs=2) as otp,
            tc.tile_pool(name="ocp", bufs=2) as ocp,
            tc.tile_pool(name="pPR", bufs=1, space="PSUM") as pPR,
            tc.tile_pool(name="pSC", bufs=4, space="PSUM") as pSC,
            tc.tile_pool(name="pOP", bufs=2, space="PSUM") as pOP,
        ):
            # ---- constants (tiny, engine-local) ----
            ident = consts.tile([128, 128], bf16)
            make_identity(nc, ident)
            eps_t = consts.tile([128, 1], f32)
            nc.vector.memset(eps_t, EPS)

            # ---- resident weights: split DMAs on scalar HWDGE queue ----
            wq_sb = bigp.tile([128, HT, DG], bf16, tag="wq")
            wqr = wqT.rearrange("(t p) d -> p t d", p=128)
            for c in range(4):
                nc.scalar.dma_start(wq_sb[:, 4 * c:4 * c + 4, :],
                                    wqr[:, 4 * c:4 * c + 4, :])
            wkv_sb = bigp.tile([128, HT, 2 * D], bf16, tag="wkv")
            wkvr = wkvT.rearrange("(t p) d -> p t d", p=128)
            for c in range(2):
                nc.scalar.dma_start(wkv_sb[:, 8 * c:8 * c + 8, :],
                                    wkvr[:, 8 * c:8 * c + 8, :])
            cmw_sb = consts.tile([128, NQC, DG], bf16)  # wide causal masks
            nc.scalar.dma_start(cmw_sb, cmw.rearrange("p (k q) -> p k q", k=NQC))
            wo_sb = bigp.tile([128, HG, HID], bf16, tag="wo")
            wor = woT.rearrange("(h p) n -> p h n", p=128)

            qT_all = bigp.tile([128, HG, S], bf16, tag="qT")   # [d, h, s]
            kT_sb = bigp.tile([128, S], bf16, tag="kT")        # [d, s]
            vvb = bigp.tile([128, NT, 132], bf16, tag="vv")    # [s%128, s//128, d|ones]
            nc.vector.memset(vvb[:, :, 128:132], 1.0)

            def emit_st(st):
                # x blocked [p, st, t, s]: per-partition 2KB contiguous runs
                xs0 = xsp.tile([128, HT // 2, 128], bf16, tag="xs")
                nc.sync.dma_start(xs0, xT[:, st, 0:HT // 2, :])
                xs1 = xsp.tile([128, HT // 2, 128], bf16, tag="xs")
                nc.sync.dma_start(xs1, xT[:, st, HT // 2:HT, :])
                xhalves = (xs0, xs1)

                cs_t = csp.tile([128, 128], f32, tag="cs")
                nc.sync.dma_start(cs_t, csx[:, st, :])
                sn_t = csp.tile([128, 128], f32, tag="sn")
                nc.sync.dma_start(sn_t, snx[:, st, :])

                if st == 1:
                    # wo not needed until the first out-projection
                    for c in range(2):
                        nc.scalar.dma_start(wo_sb[:, 2 * c:2 * c + 2, :],
                                            wor[:, 2 * c:2 * c + 2, :])

                qp = pPR.tile([128, DG], f32, tag="qp")
                for t in range(HT):
                    nc.tensor.matmul(
                        qp, lhsT=xhalves[t // 8][:, t % 8, :],
                        rhs=wq_sb[:, t, :], start=(t == 0), stop=(t == HT - 1),
                    )
                kvp = pPR.tile([128, 2 * D], f32, tag="kvp")
                for t in range(HT):
                    nc.tensor.matmul(
                        kvp, lhsT=xhalves[t // 8][:, t % 8, :],
                        rhs=wkv_sb[:, t, :], start=(t == 0), stop=(t == HT - 1),
                    )

                # stage kv psum -> SBUF (GPSIMD cannot read PSUM)
                kvs = rsp.tile([128, 2 * D], f32, tag="kvs")
                nc.scalar.copy(kvs, kvp)
                nc.vector.tensor_copy(vvb[:, st, 0:128], kvs[:, D:2 * D])

                # ---- RoPE + RMS-norm for 4 q heads, batched ----
                qs = rsp.tile([128, DG], f32, tag="qs")
                nc.scalar.copy(qs, qp)
                q3 = qs.rearrange("p (h d) -> p h d", h=HG)
                q4 = qs.rearrange("p (h t d) -> p h t d", h=HG, t=2)
                rot = rsp.tile([128, DG], f32, tag="rot")
                r4 = rot.rearrange("p (h t d) -> p h t d", h=HG, t=2)
                r3 = rot.rearrange("p (h d) -> p h d", h=HG)
                nc.gpsimd.tensor_copy(r4[:, :, 0, :], q4[:, :, 1, :])
                nc.gpsimd.tensor_scalar_mul(r4[:, :, 1, :], q4[:, :, 0, :], -1.0)
                t1 = rsp.tile([128, DG], f32, tag="t1")
                t3 = t1.rearrange("p (h d) -> p h d", h=HG)
                nc.vector.tensor_mul(t3, q3, _bc_free(cs_t, HG))
                nc.vector.tensor_mul(r3, r3, _bc_free(sn_t, HG))
                nc.vector.tensor_add(t1, t1, rot)          # t1 = roped q
                ms4 = smp.tile([128, HG], f32, tag="ms4")
                nc.vector.tensor_mul(rot, t1, t1)          # rot dead; reuse as q^2
                nc.vector.tensor_reduce(
                    ms4, rot.rearrange("p (h d) -> p h d", h=HG),
                    axis=mybir.AxisListType.X, op=add)
                srt4 = smp.tile([128, HG], f32, tag="srt4")
                nc.scalar.activation(out=srt4, in_=ms4, func=Sqrt,
                                     bias=eps_t[:, 0:1], scale=1.0 / D)
                nc.vector.reciprocal(srt4, srt4)
                # normalize all 4 heads in one DVE op: qn = t1 * srt4[h]
                qn_all = qnp.tile([128, DG], bf16, tag="qn")
                nc.vector.scalar_tensor_tensor(
                    qn_all.rearrange("p (h d) -> p h d", h=HG),
                    t1.rearrange("p (h d) -> p h d", h=HG),
                    1.0,
                    bass.AP(tensor=srt4.tensor, offset=srt4.offset,
                            ap=[srt4.ap[0], srt4.ap[1], [0, 128]]),
                    mult, mult)
                for h in range(HG):
                    tp = pOP.tile([128, DG], f32, tag="op")
                    nc.tensor.transpose(
                        tp[:, 0:64].bitcast(bf16),
                        qn_all[:, h * 128:(h + 1) * 128], ident)
                    nc.vector.tensor_copy(
                        qT_all[:, h, st * 128:(st + 1) * 128],
                        tp[:, 0:64].bitcast(bf16))

                # ---- RoPE + RMS-norm for k (single kv head) ----
                kk = kvs[:, 0:D]
                k2 = kk.rearrange("p (t d) -> p t d", t=2)
                krot = rsp.tile([128, 128], f32, tag="krot")
                kr2 = krot.rearrange("p (t d) -> p t d", t=2)
                nc.gpsimd.tensor_copy(kr2[:, 0, :], k2[:, 1, :])
                nc.gpsimd.tensor_scalar_mul(kr2[:, 1, :], k2[:, 0, :], -1.0)
                kt1 = rsp.tile([128, 128], f32, tag="kt1")
                nc.vector.tensor_mul(kt1, kk, cs_t)
                nc.vector.tensor_mul(krot, krot, sn_t)
                nc.vector.tensor_add(kt1, kt1, krot)
                msk = smp.tile([128, 1], f32, tag="msk")
                nc.vector.tensor_mul(krot, kt1, kt1)       # krot dead; reuse as k^2
                nc.vector.tensor_reduce(msk, krot, axis=mybir.AxisListType.X, op=add)
                srtk = smp.tile([128, 1], f32, tag="srtk")
                nc.scalar.activation(out=srtk, in_=msk, func=Sqrt,
                                     bias=eps_t[:, 0:1], scale=1.0 / D)
                nc.vector.reciprocal(srtk, srtk)
                kn = qnp.tile([128, 128], bf16, tag="kn")
                nc.vector.tensor_scalar_mul(kn, kt1, srtk)
                tp = pOP.tile([128, DG], f32, tag="op")
                nc.tensor.transpose(tp[:, 0:64].bitcast(bf16), kn, ident)
                nc.vector.tensor_copy(
                    kT_sb[:, st * 128:(st + 1) * 128], tp[:, 0:64].bitcast(bf16))

            def emit_qc_a(qc, heads):
                """scores -> exp -> mask -> PV for a pair of heads."""
                nkt = 4 * (qc + 1)
                pts = {}
                for h in heads:
                    for kt in range(nkt):
                        qoff = max(0, kt - 4 * qc) * 128
                        sp = pSC.tile([128, DG], f32, tag="sp")
                        nc.tensor.matmul(
                            sp[:, qoff:DG],
                            lhsT=kT_sb[:, kt * 128:(kt + 1) * 128],
                            rhs=qT_all[:, h, qc * DG + qoff:(qc + 1) * DG],
                            start=True, stop=True)
                        pt = ptp.tile([128, DG], bf16, tag="pt")
                        nc.scalar.activation(
                            out=pt[:, qoff:DG], in_=sp[:, qoff:DG],
                            func=Exp, scale=ISD)
                        pts[(h, kt)] = pt
                    # causal masking of the diagonal k tiles (tri block)
                    for qtl in range(4):
                        kt = 4 * qc + qtl
                        sl = pts[(h, kt)][:, qtl * 128:(qtl + 1) * 128]
                        nc.vector.tensor_mul(
                            sl, sl, cmw_sb[:, qtl, qtl * 128:(qtl + 1) * 128])
                # probs @ [v | ones] per q tile, then normalize -> on
                for h in heads:
                    for qtl in range(4):
                        qt = 4 * qc + qtl
                        op = pOP.tile([128, DG], f32, tag="op")
                        for kt in range(qt + 1):
                            nc.tensor.matmul(
                                op[:, 0:129],
                                lhsT=pts[(h, kt)][:, qtl * 128:(qtl + 1) * 128],
                                rhs=vvb[:, kt, 0:129],
                                start=(kt == 0), stop=(kt == qt))
                        rc = smp.tile([128, 1], f32, tag="rc")
                        nc.vector.reciprocal(rc, op[:, 128:129])
                        on = qnp.tile([128, 128], bf16, tag="on", bufs=20)
                        nc.vector.tensor_scalar_mul(on, op[:, 0:128], rc)
                        ons[(h, qtl)] = on

            def emit_qc_b(qc):
                """transposes -> out projection -> bf16 store."""
                otile = otp.tile([128, HG, DG], bf16, tag="ot")  # [d, h, q]
                for qtl in range(4):
                    for h in range(HG):
                        tp = pOP.tile([128, DG], f32, tag="op")
                        nc.tensor.transpose(
                            tp[:, 0:64].bitcast(bf16), ons.pop((h, qtl)), ident)
                        nc.vector.tensor_copy(
                            otile[:, h, qtl * 128:(qtl + 1) * 128],
                            tp[:, 0:64].bitcast(bf16))
                for stl in range(4):
                    srow = (4 * qc + stl) * 128
                    ocw = ocp.tile([128, HID], bf16, tag="ocw")
                    for cc in range(4):
                        wop = pOP.tile([128, DG], f32, tag="op")
                        for h2 in range(HG):
                            nc.tensor.matmul(
                                wop,
                                lhsT=otile[:, h2, stl * 128:(stl + 1) * 128],
                                rhs=wo_sb[:, h2, cc * DG:(cc + 1) * DG],
                                start=(h2 == 0), stop=(h2 == HG - 1))
                        if cc % 2 == 0:
                            nc.vector.tensor_copy(ocw[:, cc * DG:(cc + 1) * DG], wop)
                        else:
                            nc.scalar.copy(ocw[:, cc * DG:(cc + 1) * DG], wop)
                    if qc < 3:
                        nc.gpsimd.dma_start(out[srow:srow + 128, :], ocw)
                    else:
                        nc.sync.dma_start(out[srow:srow + 128, :], ocw)

            ons = {}
            # schedule: chunk qc's scores/PV split into head pairs (A1 after
            # stile 4qc+3, A2 after stile 4qc+4) and the back-half B after
            # stile 4qc+5, so solid stile matmuls fill the exp-paced bubbles.
            for st in range(PH1_TILES):
                emit_st(st)
                qc = st // 4
                if st % 4 == 3 and qc < PH2_CHUNKS:
                    emit_qc_a(qc, (0, 1))
                elif st % 4 == 0 and st > 0 and qc - 1 < PH2_CHUNKS:
                    emit_qc_a(qc - 1, (2, 3))
                elif st % 4 == 1 and st > 1 and qc - 1 < PH2_CHUNKS:
                    emit_qc_b(qc - 1)
            if PH1_TILES == NT and PH2_CHUNKS == NQC:
                emit_qc_a(3, (2, 3))
                emit_qc_b(3)


def _build():
    nc = bacc.Bacc("TRN2", target_bir_lowering=False, debug=False,
                   num_devices=NCORES)
    # x blocked [p, st, t, s]; cos/sin blocked [p, st, d]
    xT = nc.dram_tensor("xT", [128, NT, HT, 128], bf16, kind="ExternalInput").ap()
    wqT = nc.dram_tensor("wqT", [HID, DG], bf16, kind="ExternalInput").ap()
    wkvT = nc.dram_tensor("wkvT", [HID, 2 * D], bf16, kind="ExternalInput").ap()
    woT = nc.dram_tensor("woT", [DG, HID], bf16, kind="ExternalInput").ap()
    csx = nc.dram_tensor("csx", [128, NT, 128], f32, kind="ExternalInput").ap()
    snx = nc.dram_tensor("snx", [128, NT, 128], f32, kind="ExternalInput").ap()
    cmw = nc.dram_tensor("cmw", [128, NQC * DG], bf16, kind="ExternalInput").ap()
    out = nc.dram_tensor("out", [S, HID], bf16, kind="ExternalOutput").ap()
    _emit(nc, xT, wqT, wkvT, woT, csx, snx, cmw, out)
    nc.compile()
    return nc


def _get_compiled():
    global _compiled
    if _compiled is None:
        _compiled = _build()
    return _compiled


def _causal_masks():
    """cmw[k, ktl, q]: per diagonal-position wide mask over a 512-q chunk."""
    m = np.zeros((128, NQC, DG), np.float32)
    tri = np.triu(np.ones((128, 128), np.float32))  # 1 where k <= q
    for ktl in range(4):
        for qt in range(4):
            blk = m[:, ktl, qt * 128:(qt + 1) * 128]
            if qt > ktl:
                blk[:] = 1.0
            elif qt == ktl:
                blk[:] = tri
    return np.ascontiguousarray(
        m.reshape(128, NQC * DG).astype(ml_dtypes.bfloat16))


def kernel(x, cos, sin, wq, wk, wv, wo):
    nc = _get_compiled()
    x = np.asarray(x, np.float32)
    cos = np.asarray(cos, np.float32)
    sin = np.asarray(sin, np.float32)
    wq = np.asarray(wq, np.float32)
    wk = np.asarray(wk, np.float32)
    wv = np.asarray(wv, np.float32)
    wo = np.asarray(wo, np.float32)

    bf = ml_dtypes.bfloat16
    wkvT = np.ascontiguousarray(np.concatenate([wk, wv], 0).T.astype(bf))
    # cos/sin duplicated halves, blocked [p, st, d]
    csw = np.concatenate([cos, cos], 1).reshape(NT, 128, 128)
    snw = np.concatenate([sin, sin], 1).reshape(NT, 128, 128)
    csx = np.ascontiguousarray(csw.transpose(1, 0, 2))
    snx = np.ascontiguousarray(snw.transpose(1, 0, 2))
    cmw = _causal_masks()
    # x blocked [p, st, t, s]: xT[p, st, t, s] = x[b].T[t*128+p, st*128+s]
    xTs = []
    for b in range(B):
        xt = x[b].T.astype(bf).reshape(HT, 128, NT, 128)
        xTs.append(np.ascontiguousarray(xt.transpose(1, 2, 0, 3)))
    wqTs = [np.ascontiguousarray(wq[g * DG:(g + 1) * DG].T.astype(bf))
            for g in range(GROUPS)]
    woTs = [np.ascontiguousarray(wo[:, g * DG:(g + 1) * DG].T.astype(bf))
            for g in range(GROUPS)]

    in_maps = []
    for c in range(NCORES):
        b, g = divmod(c, GROUPS)
        in_maps.append({
            "xT": xTs[b], "wqT": wqTs[g], "wkvT": wkvT, "woT": woTs[g],
            "csx": csx, "snx": snx, "cmw": cmw,
        })
    res = run_bass_kernel_spmd(nc, in_maps, list(range(NCORES)), trace=TRACE)
    LAST["res"] = res
    outs = [r["out"] for r in res.results]
    final = np.empty((B, S, HID), np.float32)
    for b in range(B):
        final[b] = (outs[GROUPS * b].astype(np.float32)
                    + outs[GROUPS * b + 1].astype(np.float32)
                    + outs[GROUPS * b + 2].astype(np.float32)
                    + outs[GROUPS * b + 3].astype(np.float32))
    return final


# revision 5
# speedup vs baseline: 1.2477x; 1.2021x over previous
"""Causal MQA self-attention (RoPE + RMS-norm on q/k) on 8 TRN2 NeuronCores.

Sharding: core c -> (batch b = c//4, head-group g = c%4 of 4 heads).
Each core computes, for its batch and its 4 heads:
  q/k/v projections -> RoPE -> RMS-norm -> causal attention -> partial
  output projection out_part = attn_out_g @ wo[:, g].T  (shape [S, HID]).
Host sums the 4 per-group partials of each batch (row-parallel matmul
unshard) and stacks the 2 batches.

v3 over the baseline:
  - weight DMAs split into chunks on the scalar HWDGE queue; x/cos/sin
    on the sync queue -> first matmul ~2us in (was ~34us).
  - x host-repacked into [p, st, t, s] blocks: 2KB contiguous runs.
  - RoPE without rotate-copies: sin is stored [sin | -sin] (host) and
    the rotate term is built by two half-width DVE muls that read the
    projection PSUM directly (no PSUM->SBUF staging for q, k at all).
  - k's RMS-norm is folded into the softmax exp as a per-k-partition
    scale AP (kinv); q's norm carries the 1/sqrt(D) factor.
  - the 5 per-stile transposes (4 q heads + k) land in one PSUM bank
    and drain with one wide scalar-engine copy each for q / k.
  - attention chunks split A1/A2 (head pairs) + deferred B half
    (transposes/out-proj/store), interleaved between stiles.
  - diagonal score matmuls narrowed to the unmasked span; output is
    stored bf16 as whole rows; gpsimd does nothing but stores.
"""

import ml_dtypes
import numpy as np

import concourse.bass as bass
import concourse.mybir as mybir
import concourse.tile as tile
from concourse import bacc
from concourse.bass_utils import run_bass_kernel_spmd
from concourse.masks import make_identity

# problem dims (hardcoded per contract)
B, S, HID, H, D = 2, 2048, 2048, 16, 128
NCORES = 8
GROUPS = 4              # head-groups = cores per batch
HG = H // GROUPS        # heads per core
DG = HG * D             # 512 projected q dims per core
NT = S // 128           # 16 sequence tiles
HT = HID // 128         # 16 hidden tiles
NQC = 4                 # q chunks of 512 columns
EPS = 1.1920928955078125e-07
ISD = 1.0 / float(np.sqrt(D))

f32 = mybir.dt.float32
bf16 = mybir.dt.bfloat16

TRACE = False           # test harness may flip this for NTFF profiling
LAST = {}               # last BassKernelResults, for the test harness
PH1_TILES = NT          # bisect knob
PH2_CHUNKS = NQC        # bisect knob

_compiled = None


def _bc(src, n):
    """[128, F] view -> [128, n, F] AP broadcasting along a step-0 mid dim."""
    return bass.AP(
        tensor=src.tensor,
        offset=src.offset,
        ap=[src.ap[0], [0, n], src.ap[1]],
    )


def _emit(nc, xT, wqT, wkvT, woT, csx, snx, cmw, out):
    add = mybir.AluOpType.add
    mult = mybir.AluOpType.mult
    Sqrt = mybir.ActivationFunctionType.Sqrt
    Exp = mybir.ActivationFunctionType.Exp

    with tile.TileContext(nc) as tc:
        with (
            tc.tile_pool(name="consts", bufs=1) as consts,
            tc.tile_pool(name="bigp", bufs=1) as bigp,
            tc.tile_pool(name="xsp", bufs=6) as xsp,
            tc.tile_pool(name="csp", bufs=3) as csp,
            tc.tile_pool(name="rsp", bufs=2) as rsp,
            tc.tile_pool(name="smp", bufs=4) as smp,
            tc.tile_pool(name="qnp", bufs=4) as qnp,
            tc.tile_pool(name="ptp", bufs=34) as ptp,
            tc.tile_pool(name="otp", bufs=2) as otp,
            tc.tile_pool(name="ocp", bufs=2) as ocp,
            tc.tile_pool(name="pPR", bufs=1, space="PSUM") as pPR,
            tc.tile_pool(name="pSC", bufs=4, space="PSUM") as pSC,
            tc.tile_pool(name="pOP", bufs=2, space="PSUM") as pOP,
        ):
            # ---- constants (tiny, engine-local) ----
            ident = consts.tile([128, 128], bf16)
            make_identity(nc, ident)
            eps_t = consts.tile([128, 1], f32)
            nc.vector.memset(eps_t, EPS)

            # ---- resident weights: split DMAs on scalar HWDGE queue ----
            wq_sb = bigp.tile([128, HT, DG], bf16, tag="wq")
            wqr = wqT.rearrange("(t p) d -> p t d", p=128)
            for c in range(4):
                nc.scalar.dma_start(wq_sb[:, 4 * c:4 * c + 4, :],
                                    wqr[:, 4 * c:4 * c + 4, :])
            wkv_sb = bigp.tile([128, HT, 2 * D], bf16, tag="wkv")
            wkvr = wkvT.rearrange("(t p) d -> p t d", p=128)
            for c in range(2):
                nc.scalar.dma_start(wkv_sb[:, 8 * c:8 * c + 8, :],
                                    wkvr[:, 8 * c:8 * c + 8, :])
            cmw_sb = consts.tile([128, NQC, DG], bf16)  # wide causal masks
            nc.scalar.dma_start(cmw_sb, cmw.rearrange("p (k q) -> p k q", k=NQC))
            wo_sb = bigp.tile([128, HG, HID], bf16, tag="wo")
            wor = woT.rearrange("(h p) n -> p h n", p=128)

            qT_all = bigp.tile([128, HG, S], bf16, tag="qT")   # [d, h, s]
            kT_sb = bigp.tile([128, S], bf16, tag="kT")        # [d, s]
            vvb = bigp.tile([128, NT, 132], bf16, tag="vv")    # [s%128, s//128, d|ones]
            nc.vector.memset(vvb[:, :, 128:132], 1.0)
            kinv_all = bigp.tile([128, NT], f32, tag="kinv")   # per-k exp scales

            def emit_st(st):
                # x blocked [p, st, t, s]: per-partition 2KB contiguous runs
                xs0 = xsp.tile([128, HT // 2, 128], bf16, tag="xs")
                nc.sync.dma_start(xs0, xT[:, st, 0:HT // 2, :])
                xs1 = xsp.tile([128, HT // 2, 128], bf16, tag="xs")
                nc.sync.dma_start(xs1, xT[:, st, HT // 2:HT, :])
                xhalves = (xs0, xs1)

                cs_t = csp.tile([128, 128], f32, tag="cs")
                nc.sync.dma_start(cs_t, csx[:, st, :])
                sn_t = csp.tile([128, 128], f32, tag="sn")   # [sin | -sin]
                nc.sync.dma_start(sn_t, snx[:, st, :])

                if st == 2:
                    # wo not needed until the first out-projection
                    for c in range(2):
                        nc.scalar.dma_start(wo_sb[:, 2 * c:2 * c + 2, :],
                                            wor[:, 2 * c:2 * c + 2, :])

                qp = pPR.tile([128, DG], f32, tag="qp")
                for t in range(HT):
                    nc.tensor.matmul(
                        qp, lhsT=xhalves[t // 8][:, t % 8, :],
                        rhs=wq_sb[:, t, :], start=(t == 0), stop=(t == HT - 1),
                    )
                kvp = pPR.tile([128, 2 * D], f32, tag="kvp")
                for t in range(HT):
                    nc.tensor.matmul(
                        kvp, lhsT=xhalves[t // 8][:, t % 8, :],
                        rhs=wkv_sb[:, t, :], start=(t == 0), stop=(t == HT - 1),
                    )
                nc.scalar.copy(vvb[:, st, 0:128], kvp[:, D:2 * D])

                # ---- RoPE + RMS-norm for 4 q heads, batched ----
                # rot = [q_hi * sin, q_lo * -sin] via half-width muls that
                # read the projection PSUM directly (fp32 DVE has no 2x
                # mode to lose); t1 = q * cos + rot.
                q3 = qp.rearrange("p (h d) -> p h d", h=HG)
                q4 = qp.rearrange("p (h t d) -> p h t d", h=HG, t=2)
                rot = rsp.tile([128, DG], f32, tag="rot")
                r4 = rot.rearrange("p (h t d) -> p h t d", h=HG, t=2)
                nc.vector.tensor_mul(r4[:, :, 0, :], q4[:, :, 1, :],
                                     _bc(sn_t[:, 0:64], HG))
                nc.vector.tensor_mul(r4[:, :, 1, :], q4[:, :, 0, :],
                                     _bc(sn_t[:, 64:128], HG))
                t1 = rsp.tile([128, DG], f32, tag="t1")
                t3 = t1.rearrange("p (h d) -> p h d", h=HG)
                nc.vector.tensor_mul(t3, q3, _bc(cs_t, HG))
                nc.vector.tensor_add(t1, t1, rot)          # t1 = roped q
                ms4 = smp.tile([128, HG], f32, tag="ms4")
                nc.vector.tensor_mul(rot, t1, t1)          # rot dead; reuse as q^2
                nc.vector.tensor_reduce(
                    ms4, rot.rearrange("p (h d) -> p h d", h=HG),
                    axis=mybir.AxisListType.X, op=add)
                srt4 = smp.tile([128, HG], f32, tag="srt4")
                nc.scalar.activation(out=srt4, in_=ms4, func=Sqrt,
                                     bias=eps_t[:, 0:1], scale=1.0 / D)
                nc.vector.reciprocal(srt4, srt4)
                # qn = t1 * ISD * srt4[h]  (ISD rides here; k-norm rides exp)
                qn_all = qnp.tile([128, DG], bf16, tag="qn")
                nc.vector.scalar_tensor_tensor(
                    qn_all.rearrange("p (h d) -> p h d", h=HG),
                    t1.rearrange("p (h d) -> p h d", h=HG),
                    ISD,
                    bass.AP(tensor=srt4.tensor, offset=srt4.offset,
                            ap=[srt4.ap[0], srt4.ap[1], [0, 128]]),
                    mult, mult)

                # ---- RoPE for k (single kv head); norm deferred to exp ----
                kk = kvp[:, 0:D]
                k2 = kk.rearrange("p (t d) -> p t d", t=2)
                krot = rsp.tile([128, 128], f32, tag="krot")
                kr2 = krot.rearrange("p (t d) -> p t d", t=2)
                nc.vector.tensor_mul(kr2[:, 0, :], k2[:, 1, :], sn_t[:, 0:64])
                nc.vector.tensor_mul(kr2[:, 1, :], k2[:, 0, :], sn_t[:, 64:128])
                kt1 = rsp.tile([128, 128], f32, tag="kt1")
                nc.vector.tensor_mul(kt1, kk, cs_t)
                nc.vector.tensor_add(kt1, kt1, krot)
                msk = smp.tile([128, 1], f32, tag="msk")
                nc.vector.tensor_mul(krot, kt1, kt1)       # krot dead; reuse as k^2
                nc.vector.tensor_reduce(msk, krot, axis=mybir.AxisListType.X, op=add)
                srtk = smp.tile([128, 1], f32, tag="srtk")
                nc.scalar.activation(out=srtk, in_=msk, func=Sqrt,
                                     bias=eps_t[:, 0:1], scale=1.0 / D)
                nc.vector.reciprocal(kinv_all[:, st:st + 1], srtk)
                kn = qnp.tile([128, 128], bf16, tag="kn")
                nc.vector.tensor_copy(kn, kt1)             # cast only

                # ---- 5 transposes into one PSUM bank, drained by 2 copies
                opt = pOP.tile([128, DG], f32, tag="op")
                opt_bf = opt.bitcast(bf16)                 # [128, 1024] bf16
                for h in range(HG):
                    nc.tensor.transpose(
                        opt_bf[:, h * 128:(h + 1) * 128],
                        qn_all[:, h * 128:(h + 1) * 128], ident)
                nc.tensor.transpose(opt_bf[:, 512:640], kn, ident)
                nc.scalar.copy(
                    qT_all[:, :, st * 128:(st + 1) * 128],
                    opt_bf[:, 0:512].rearrange("p (h s) -> p h s", h=HG))
                nc.scalar.copy(
                    kT_sb[:, st * 128:(st + 1) * 128], opt_bf[:, 512:640])

            def emit_qc_a(qc, heads):
                """scores -> exp (with per-k kinv scale) -> mask -> PV."""
                nkt = 4 * (qc + 1)
                pts = {}
                for h in heads:
                    for kt in range(nkt):
                        qoff = max(0, kt - 4 * qc) * 128
                        sp = pSC.tile([128, DG], f32, tag="sp")
                        nc.tensor.matmul(
                            sp[:, qoff:DG],
                            lhsT=kT_sb[:, kt * 128:(kt + 1) * 128],
                            rhs=qT_all[:, h, qc * DG + qoff:(qc + 1) * DG],
                            start=True, stop=True)
                        pt = ptp.tile([128, DG], bf16, tag="pt")
                        nc.scalar.activation(
                            out=pt[:, qoff:DG], in_=sp[:, qoff:DG],
                            func=Exp, scale=kinv_all[:, kt:kt + 1])
                        pts[(h, kt)] = pt
                    # causal masking of the diagonal k tiles (tri block)
                    for qtl in range(4):
                        kt = 4 * qc + qtl
                        sl = pts[(h, kt)][:, qtl * 128:(qtl + 1) * 128]
                        nc.vector.tensor_mul(
                            sl, sl, cmw_sb[:, qtl, qtl * 128:(qtl + 1) * 128])
                # probs @ [v | ones] per q tile, then normalize -> on
                for h in heads:
                    for qtl in range(4):
                        qt = 4 * qc + qtl
                        op = pOP.tile([128, DG], f32, tag="op")
                        for kt in range(qt + 1):
                            nc.tensor.matmul(
                                op[:, 0:129],
                                lhsT=pts[(h, kt)][:, qtl * 128:(qtl + 1) * 128],
                                rhs=vvb[:, kt, 0:129],
                                start=(kt == 0), stop=(kt == qt))
                        rc = smp.tile([128, 1], f32, tag="rc")
                        nc.vector.reciprocal(rc, op[:, 128:129])
                        on = qnp.tile([128, 128], bf16, tag="on", bufs=20)
                        nc.vector.tensor_scalar_mul(on, op[:, 0:128], rc)
                        ons[(h, qtl)] = on

            def emit_qc_b(qc):
                """transposes -> out projection -> bf16 store."""
                otile = otp.tile([128, HG, DG], bf16, tag="ot")  # [d, h, q]
                for qtl in range(4):
                    opt = pOP.tile([128, DG], f32, tag="op")
                    opt_bf = opt.bitcast(bf16)
                    for h in range(HG):
                        nc.tensor.transpose(
                            opt_bf[:, h * 128:(h + 1) * 128],
                            ons.pop((h, qtl)), ident)
                    nc.scalar.copy(
                        otile[:, :, qtl * 128:(qtl + 1) * 128],
                        opt_bf[:, 0:512].rearrange("p (h s) -> p h s", h=HG))
                for stl in range(4):
                    srow = (4 * qc + stl) * 128
                    ocw = ocp.tile([128, HID], bf16, tag="ocw")
                    for cc in range(4):
                        wop = pOP.tile([128, DG], f32, tag="op")
                        for h2 in range(HG):
                            nc.tensor.matmul(
                                wop,
                                lhsT=otile[:, h2, stl * 128:(stl + 1) * 128],
                                rhs=wo_sb[:, h2, cc * DG:(cc + 1) * DG],
                                start=(h2 == 0), stop=(h2 == HG - 1))
                        if cc % 2 == 0:
                            nc.vector.tensor_copy(ocw[:, cc * DG:(cc + 1) * DG], wop)
                        else:
                            nc.scalar.copy(ocw[:, cc * DG:(cc + 1) * DG], wop)
                    if qc < 3:
                        nc.gpsimd.dma_start(out[srow:srow + 128, :], ocw)
                    else:
                        nc.sync.dma_start(out[srow:srow + 128, :], ocw)

            ons = {}
            # schedule: chunk qc's scores/PV split into head pairs (A1 after
            # stile 4qc+3, A2 after stile 4qc+4) and the back-half B after
            # stile 4qc+5, so solid stile matmuls fill the exp-paced bubbles.
            for st in range(PH1_TILES):
                emit_st(st)
                qc = st // 4
                if st % 4 == 3 and qc < PH2_CHUNKS:
                    emit_qc_a(qc, (0, 1))
                elif st % 4 == 0 and st > 0 and qc - 1 < PH2_CHUNKS:
                    emit_qc_a(qc - 1, (2, 3))
                elif st % 4 == 1 and st > 1 and qc - 1 < PH2_CHUNKS:
                    emit_qc_b(qc - 1)
            if PH1_TILES == NT and PH2_CHUNKS == NQC:
                emit_qc_a(3, (2, 3))
                emit_qc_b(3)


def _build():
    nc = bacc.Bacc("TRN2", target_bir_lowering=False, debug=False,
                   num_devices=NCORES)
    # x blocked [p, st, t, s]; cos/sin blocked [p, st, d]
    xT = nc.dram_tensor("xT", [128, NT, HT, 128], bf16, kind="ExternalInput").ap()
    wqT = nc.dram_tensor("wqT", [HID, DG], bf16, kind="ExternalInput").ap()
    wkvT = nc.dram_tensor("wkvT", [HID, 2 * D], bf16, kind="ExternalInput").ap()
    woT = nc.dram_tensor("woT", [DG, HID], bf16, kind="ExternalInput").ap()
    csx = nc.dram_tensor("csx", [128, NT, 128], f32, kind="ExternalInput").ap()
    snx = nc.dram_tensor("snx", [128, NT, 128], f32, kind="ExternalInput").ap()
    cmw = nc.dram_tensor("cmw", [128, NQC * DG], bf16, kind="ExternalInput").ap()
    out = nc.dram_tensor("out", [S, HID], bf16, kind="ExternalOutput").ap()
    _emit(nc, xT, wqT, wkvT, woT, csx, snx, cmw, out)
    nc.compile()
    return nc


def _get_compiled():
    global _compiled
    if _compiled is None:
        _compiled = _build()
    return _compiled


def _causal_masks():
    """cmw[k, ktl, q]: per diagonal-position wide mask over a 512-q chunk."""
    m = np.zeros((128, NQC, DG), np.float32)
    tri = np.triu(np.ones((128, 128), np.float32))  # 1 where k <= q
    for ktl in range(4):
        for qt in range(4):
            blk = m[:, ktl, qt * 128:(qt + 1) * 128]
            if qt > ktl:
                blk[:] = 1.0
            elif qt == ktl:
                blk[:] = tri
    return np.ascontiguousarray(
        m.reshape(128, NQC * DG).astype(ml_dtypes.bfloat16))


def kernel(x, cos, sin, wq, wk, wv, wo):
    nc = _get_compiled()
    x = np.asarray(x, np.float32)
    cos = np.asarray(cos, np.float32)
    sin = np.asarray(sin, np.float32)
    wq = np.asarray(wq, np.float32)
    wk = np.asarray(wk, np.float32)
    wv = np.asarray(wv, np.float32)
    wo = np.asarray(wo, np.float32)

    bf = ml_dtypes.bfloat16
    wkvT = np.ascontiguousarray(np.concatenate([wk, wv], 0).T.astype(bf))
    # cos duplicated halves; sin stored [sin | -sin]; blocked [p, st, d]
    csw = np.concatenate([cos, cos], 1).reshape(NT, 128, 128)
    snw = np.concatenate([sin, -sin], 1).reshape(NT, 128, 128)
    csx = np.ascontiguousarray(csw.transpose(1, 0, 2))
    snx = np.ascontiguousarray(snw.transpose(1, 0, 2))
    cmw = _causal_masks()
    # x blocked [p, st, t, s]: xT[p, st, t, s] = x[b].T[t*128+p, st*128+s]
    xTs = []
    for b in range(B):
        xt = x[b].T.astype(bf).reshape(HT, 128, NT, 128)
        xTs.append(np.ascontiguousarray(xt.transpose(1, 2, 0, 3)))
    wqTs = [np.ascontiguousarray(wq[g * DG:(g + 1) * DG].T.astype(bf))
            for g in range(GROUPS)]
    woTs = [np.ascontiguousarray(wo[:, g * DG:(g + 1) * DG].T.astype(bf))
            for g in range(GROUPS)]

    in_maps = []
    for c in range(NCORES):
        b, g = divmod(c, GROUPS)
        in_maps.append({
            "xT": xTs[b], "wqT": wqTs[g], "wkvT": wkvT, "woT": woTs[g],
            "csx": csx, "snx": snx, "cmw": cmw,
        })
    res = run_bass_kernel_spmd(nc, in_maps, list(range(NCORES)), trace=TRACE)
    LAST["res"] = res
    outs = [r["out"] for r in res.results]
    final = np.empty((B, S, HID), np.float32)
    for b in range(B):
        final[b] = (outs[GROUPS * b].astype(np.float32)
                    + outs[GROUPS * b + 1].astype(np.float32)
                    + outs[GROUPS * b + 2].astype(np.float32)
                    + outs[GROUPS * b + 3].astype(np.float32))
    return final


# revision 15
# speedup vs baseline: 1.3062x; 1.0469x over previous
"""Causal MQA self-attention (RoPE + RMS-norm on q/k) on 8 TRN2 NeuronCores.

Sharding: core c -> (batch b = c//4, head-group g = c%4 of 4 heads).
Each core computes, for its batch and its 4 heads:
  q/k/v projections -> RoPE -> RMS-norm -> causal attention -> partial
  output projection out_part = attn_out_g @ wo[:, g].T  (shape [S, HID]).
Host sums the 4 per-group partials of each batch (row-parallel matmul
unshard) and stacks the 2 batches.

v3 over the baseline:
  - weight DMAs split into chunks on the scalar HWDGE queue; x/cos/sin
    on the sync queue -> first matmul ~2us in (was ~34us).
  - x host-repacked into [p, st, t, s] blocks: 2KB contiguous runs.
  - RoPE without rotate-copies: sin is stored [sin | -sin] (host) and
    the rotate term is built by two half-width DVE muls that read the
    projection PSUM directly (no PSUM->SBUF staging for q, k at all).
  - k's RMS-norm is folded into the softmax exp as a per-k-partition
    scale AP (kinv); q's norm carries the 1/sqrt(D) factor.
  - the 5 per-stile transposes (4 q heads + k) land in one PSUM bank
    and drain with one wide scalar-engine copy each for q / k.
  - attention chunks split A1/A2 (head pairs) + deferred B half
    (transposes/out-proj/store), interleaved between stiles.
  - diagonal score matmuls narrowed to the unmasked span; output is
    stored bf16 as whole rows; gpsimd does nothing but stores.
"""

import ml_dtypes
import numpy as np

import concourse.bass as bass
import concourse.mybir as mybir
import concourse.tile as tile
from concourse import bacc
from concourse.bass_utils import run_bass_kernel_spmd
from concourse.masks import make_identity

# problem dims (hardcoded per contract)
B, S, HID, H, D = 2, 2048, 2048, 16, 128
NCORES = 8
GROUPS = 4              # head-groups = cores per batch
HG = H // GROUPS        # heads per core
DG = HG * D             # 512 projected q dims per core
NT = S // 128           # 16 sequence tiles
HT = HID // 128         # 16 hidden tiles
NQC = 4                 # q chunks of 512 columns
EPS = 1.1920928955078125e-07
ISD = 1.0 / float(np.sqrt(D))

f32 = mybir.dt.float32
bf16 = mybir.dt.bfloat16

TRACE = False           # test harness may flip this for NTFF profiling
LAST = {}               # last BassKernelResults, for the test harness
PH1_TILES = NT          # bisect knob
PH2_CHUNKS = NQC        # bisect knob

_compiled = None


def _bc(src, n):
    """[128, F] view -> [128, n, F] AP broadcasting along a step-0 mid dim."""
    return bass.AP(
        tensor=src.tensor,
        offset=src.offset,
        ap=[src.ap[0], [0, n], src.ap[1]],
    )


def _emit(nc, xT, wqT, wkvT, woT, csx, snx, cmw, out):
    add = mybir.AluOpType.add
    mult = mybir.AluOpType.mult
    Sqrt = mybir.ActivationFunctionType.Sqrt
    Exp = mybir.ActivationFunctionType.Exp

    with tile.TileContext(nc) as tc:
        with (
            tc.tile_pool(name="consts", bufs=1) as consts,
            tc.tile_pool(name="bigp", bufs=1) as bigp,
            tc.tile_pool(name="xsp", bufs=6) as xsp,
            tc.tile_pool(name="csp", bufs=3) as csp,
            tc.tile_pool(name="rsp", bufs=2) as rsp,
            tc.tile_pool(name="smp", bufs=4) as smp,
            tc.tile_pool(name="qnp", bufs=4) as qnp,
            tc.tile_pool(name="ptp", bufs=34) as ptp,
            tc.tile_pool(name="otp", bufs=2) as otp,
            tc.tile_pool(name="ocp", bufs=2) as ocp,
            tc.tile_pool(name="pPR", bufs=2, space="PSUM") as pPR,
            tc.tile_pool(name="pSC", bufs=3, space="PSUM") as pSC,
            tc.tile_pool(name="pOP", bufs=2, space="PSUM") as pOP,
        ):
            # ---- constants (tiny, engine-local) ----
            ident = consts.tile([128, 128], bf16)
            make_identity(nc, ident)
            eps_t = consts.tile([128, 1], f32)
            nc.vector.memset(eps_t, EPS)

            # ---- resident weights: split DMAs on scalar HWDGE queue ----
            wq_sb = bigp.tile([128, HT, DG], bf16, tag="wq")
            wqr = wqT.rearrange("(t p) d -> p t d", p=128)
            for c in range(4):
                nc.scalar.dma_start(wq_sb[:, 4 * c:4 * c + 4, :],
                                    wqr[:, 4 * c:4 * c + 4, :])
            wkv_sb = bigp.tile([128, HT, 2 * D], bf16, tag="wkv")
            wkvr = wkvT.rearrange("(t p) d -> p t d", p=128)
            for c in range(2):
                nc.scalar.dma_start(wkv_sb[:, 8 * c:8 * c + 8, :],
                                    wkvr[:, 8 * c:8 * c + 8, :])
            cmw_sb = consts.tile([128, NQC, DG], bf16)  # wide causal masks
            wo_sb = bigp.tile([128, HG, HID], bf16, tag="wo")
            wor = woT.rearrange("(h p) n -> p h n", p=128)

            qT_all = bigp.tile([128, HG, S], bf16, tag="qT")   # [d, h, s]
            kT_sb = bigp.tile([128, S], bf16, tag="kT")        # [d, s]
            vvb = bigp.tile([128, NT, 132], bf16, tag="vv")    # [s%128, s//128, d|ones]
            nc.vector.memset(vvb[:, :, 128:132], 1.0)
            kinv_all = bigp.tile([128, NT], f32, tag="kinv")   # per-k exp scales

            def emit_st_proj(st):
                # x blocked [p, st, t, s]: per-partition 2KB contiguous runs
                xs0 = xsp.tile([128, HT // 2, 128], bf16, tag="xs")
                nc.sync.dma_start(xs0, xT[:, st, 0:HT // 2, :])
                xs1 = xsp.tile([128, HT // 2, 128], bf16, tag="xs")
                nc.sync.dma_start(xs1, xT[:, st, HT // 2:HT, :])
                xhalves = (xs0, xs1)

                cs_t = csp.tile([128, 128], bf16, tag="cs")
                nc.sync.dma_start(cs_t, csx[:, st, :])
                sn_t = csp.tile([128, 128], bf16, tag="sn")   # [sin | -sin]
                nc.sync.dma_start(sn_t, snx[:, st, :])

                if st == 2:
                    nc.scalar.dma_start(
                        cmw_sb, cmw.rearrange("p (k q) -> p k q", k=NQC))
                elif st == 3:
                    # wo not needed until the first out-projection
                    for c in range(2):
                        nc.scalar.dma_start(wo_sb[:, 2 * c:2 * c + 2, :],
                                            wor[:, 2 * c:2 * c + 2, :])

                qp = pPR.tile([128, DG], f32, tag="qp")
                for t in range(HT):
                    nc.tensor.matmul(
                        qp, lhsT=xhalves[t // 8][:, t % 8, :],
                        rhs=wq_sb[:, t, :], start=(t == 0), stop=(t == HT - 1),
                    )
                kvp = pPR.tile([128, 2 * D], f32, tag="kvp", bufs=1)
                for t in range(HT):
                    nc.tensor.matmul(
                        kvp, lhsT=xhalves[t // 8][:, t % 8, :],
                        rhs=wkv_sb[:, t, :], start=(t == 0), stop=(t == HT - 1),
                    )
                nc.scalar.copy(vvb[:, st, 0:128], kvp[:, D:2 * D])
                return qp, kvp, cs_t, sn_t

            def emit_st_fin(st, staged):
                qp, kvp, cs_t, sn_t = staged
                # ---- RoPE + RMS-norm for 4 q heads, batched ----
                # rot = [q_hi * sin, q_lo * -sin] via half-width muls that
                # read the projection PSUM directly (fp32 DVE has no 2x
                # mode to lose); t1 = q * cos + rot.
                q3 = qp.rearrange("p (h d) -> p h d", h=HG)
                q4 = qp.rearrange("p (h t d) -> p h t d", h=HG, t=2)
                rot = rsp.tile([128, DG], f32, tag="rot")
                r4 = rot.rearrange("p (h t d) -> p h t d", h=HG, t=2)
                nc.vector.tensor_mul(r4[:, :, 0, :], q4[:, :, 1, :],
                                     _bc(sn_t[:, 0:64], HG))
                nc.vector.tensor_mul(r4[:, :, 1, :], q4[:, :, 0, :],
                                     _bc(sn_t[:, 64:128], HG))
                t1 = rsp.tile([128, DG], f32, tag="t1")
                t3 = t1.rearrange("p (h d) -> p h d", h=HG)
                nc.vector.tensor_mul(t3, q3, _bc(cs_t, HG))
                nc.vector.tensor_add(t1, t1, rot)          # t1 = roped q
                ms4 = smp.tile([128, HG], f32, tag="ms4")
                nc.vector.tensor_mul(rot, t1, t1)          # rot dead; reuse as q^2
                nc.vector.tensor_reduce(
                    ms4, rot.rearrange("p (h d) -> p h d", h=HG),
                    axis=mybir.AxisListType.X, op=add)
                srt4 = smp.tile([128, HG], f32, tag="srt4")
                nc.scalar.activation(out=srt4, in_=ms4, func=Sqrt,
                                     bias=eps_t[:, 0:1], scale=1.0 / D)
                nc.vector.reciprocal(srt4, srt4)
                # qn = t1 * ISD * srt4[h]  (ISD rides here; k-norm rides exp)
                qn_all = qnp.tile([128, DG], bf16, tag="qn")
                nc.vector.scalar_tensor_tensor(
                    qn_all.rearrange("p (h d) -> p h d", h=HG),
                    t1.rearrange("p (h d) -> p h d", h=HG),
                    ISD,
                    bass.AP(tensor=srt4.tensor, offset=srt4.offset,
                            ap=[srt4.ap[0], srt4.ap[1], [0, 128]]),
                    mult, mult)

                # ---- RoPE for k (single kv head); norm deferred to exp ----
                kk = kvp[:, 0:D]
                k2 = kk.rearrange("p (t d) -> p t d", t=2)
                krot = rsp.tile([128, 128], f32, tag="krot")
                kr2 = krot.rearrange("p (t d) -> p t d", t=2)
                nc.vector.tensor_mul(kr2[:, 0, :], k2[:, 1, :], sn_t[:, 0:64])
                nc.vector.tensor_mul(kr2[:, 1, :], k2[:, 0, :], sn_t[:, 64:128])
                kt1 = rsp.tile([128, 128], f32, tag="kt1")
                nc.vector.tensor_mul(kt1, kk, cs_t)
                nc.vector.tensor_add(kt1, kt1, krot)
                msk = smp.tile([128, 1], f32, tag="msk")
                nc.vector.tensor_mul(krot, kt1, kt1)       # krot dead; reuse as k^2
                nc.vector.tensor_reduce(msk, krot, axis=mybir.AxisListType.X, op=add)
                srtk = smp.tile([128, 1], f32, tag="srtk")
                nc.scalar.activation(out=srtk, in_=msk, func=Sqrt,
                                     bias=eps_t[:, 0:1], scale=1.0 / D)
                nc.vector.reciprocal(kinv_all[:, st:st + 1], srtk)
                kn = qnp.tile([128, 128], bf16, tag="kn")
                nc.vector.tensor_copy(kn, kt1)             # cast only

                # ---- 5 transposes into one PSUM bank, drained by 2 copies
                opt = pOP.tile([128, DG], f32, tag="op")
                opt_bf = opt.bitcast(bf16)                 # [128, 1024] bf16
                for h in range(HG):
                    nc.tensor.transpose(
                        opt_bf[:, h * 128:(h + 1) * 128],
                        qn_all[:, h * 128:(h + 1) * 128], ident)
                nc.tensor.transpose(opt_bf[:, 512:640], kn, ident)
                nc.scalar.copy(
                    qT_all[:, :, st * 128:(st + 1) * 128],
                    opt_bf[:, 0:512].rearrange("p (h s) -> p h s", h=HG))
                nc.scalar.copy(
                    kT_sb[:, st * 128:(st + 1) * 128], opt_bf[:, 512:640])

            def emit_qc_a(qc, heads):
                """scores -> exp (with per-k kinv scale) -> mask -> PV."""
                nkt = 4 * (qc + 1)
                pts = {}
                for h in heads:
                    for kt in range(nkt):
                        qoff = max(0, kt - 4 * qc) * 128
                        sp = pSC.tile([128, DG], f32, tag="sp")
                        nc.tensor.matmul(
                            sp[:, qoff:DG],
                            lhsT=kT_sb[:, kt * 128:(kt + 1) * 128],
                            rhs=qT_all[:, h, qc * DG + qoff:(qc + 1) * DG],
                            start=True, stop=True)
                        pt = ptp.tile([128, DG], bf16, tag="pt")
                        nc.scalar.activation(
                            out=pt[:, qoff:DG], in_=sp[:, qoff:DG],
                            func=Exp, scale=kinv_all[:, kt:kt + 1])
                        pts[(h, kt)] = pt
                    # causal masking of the diagonal k tiles (tri block)
                    for qtl in range(4):
                        kt = 4 * qc + qtl
                        sl = pts[(h, kt)][:, qtl * 128:(qtl + 1) * 128]
                        nc.vector.tensor_mul(
                            sl, sl, cmw_sb[:, qtl, qtl * 128:(qtl + 1) * 128])
                # probs @ [v | ones] per q tile, then normalize -> on
                for h in heads:
                    for qtl in range(4):
                        qt = 4 * qc + qtl
                        op = pOP.tile([128, DG], f32, tag="op")
                        for kt in range(qt + 1):
                            nc.tensor.matmul(
                                op[:, 0:129],
                                lhsT=pts[(h, kt)][:, qtl * 128:(qtl + 1) * 128],
                                rhs=vvb[:, kt, 0:129],
                                start=(kt == 0), stop=(kt == qt))
                        rc = smp.tile([128, 1], f32, tag="rc")
                        nc.vector.reciprocal(rc, op[:, 128:129])
                        on = qnp.tile([128, 128], bf16, tag="on", bufs=20)
                        nc.vector.tensor_scalar_mul(on, op[:, 0:128], rc)
                        ons[(h, qtl)] = on

            def emit_qc_b(qc):
                """transposes -> out projection -> bf16 store."""
                otile = otp.tile([128, HG, DG], bf16, tag="ot")  # [d, h, q]
                for qtl in range(4):
                    opt = pOP.tile([128, DG], f32, tag="op")
                    opt_bf = opt.bitcast(bf16)
                    for h in range(HG):
                        nc.tensor.transpose(
                            opt_bf[:, h * 128:(h + 1) * 128],
                            ons.pop((h, qtl)), ident)
                    nc.scalar.copy(
                        otile[:, :, qtl * 128:(qtl + 1) * 128],
                        opt_bf[:, 0:512].rearrange("p (h s) -> p h s", h=HG))
                for stl in range(4):
                    srow = (4 * qc + stl) * 128
                    ocw = ocp.tile([128, HID], bf16, tag="ocw")
                    for cc in range(4):
                        wop = pOP.tile([128, DG], f32, tag="op")
                        for h2 in range(HG):
                            nc.tensor.matmul(
                                wop,
                                lhsT=otile[:, h2, stl * 128:(stl + 1) * 128],
                                rhs=wo_sb[:, h2, cc * DG:(cc + 1) * DG],
                                start=(h2 == 0), stop=(h2 == HG - 1))
                        if cc % 2 == 0:
                            nc.vector.tensor_copy(ocw[:, cc * DG:(cc + 1) * DG], wop)
                        else:
                            nc.scalar.copy(ocw[:, cc * DG:(cc + 1) * DG], wop)
                    if qc < 3:
                        nc.gpsimd.dma_start(out[srow:srow + 128, :], ocw)
                    else:
                        nc.sync.dma_start(out[srow:srow + 128, :], ocw)

            ons = {}
            # schedule: stile projections run one ahead of the rope/norm
            # finish (fills the DVE-chain latency with solid matmuls), and
            # chunk qc's attention is split into head pairs A1/A2 plus a
            # deferred back-half B, each slotted between stile finishes.
            staged = {}

            def after_fin(f):
                qc = f // 4
                if f % 4 == 3 and qc < PH2_CHUNKS:
                    emit_qc_a(qc, (0, 1))
                elif f % 4 == 0 and f > 0 and qc - 1 < PH2_CHUNKS:
                    emit_qc_a(qc - 1, (2, 3))
                elif f % 4 == 1 and f > 1 and qc - 1 < PH2_CHUNKS:
                    emit_qc_b(qc - 1)

            for st in range(PH1_TILES):
                staged[st] = emit_st_proj(st)
                if st >= 1:
                    emit_st_fin(st - 1, staged.pop(st - 1))
                    after_fin(st - 1)
            last = PH1_TILES - 1
            emit_st_fin(last, staged.pop(last))
            after_fin(last)
            if PH1_TILES == NT and PH2_CHUNKS == NQC:
                emit_qc_a(3, (2, 3))
                emit_qc_b(3)


def _build():
    nc = bacc.Bacc("TRN2", target_bir_lowering=False, debug=False,
                   num_devices=NCORES)
    # x blocked [p, st, t, s]; cos/sin blocked [p, st, d]
    xT = nc.dram_tensor("xT", [128, NT, HT, 128], bf16, kind="ExternalInput").ap()
    wqT = nc.dram_tensor("wqT", [HID, DG], bf16, kind="ExternalInput").ap()
    wkvT = nc.dram_tensor("wkvT", [HID, 2 * D], bf16, kind="ExternalInput").ap()
    woT = nc.dram_tensor("woT", [DG, HID], bf16, kind="ExternalInput").ap()
    csx = nc.dram_tensor("csx", [128, NT, 128], bf16, kind="ExternalInput").ap()
    snx = nc.dram_tensor("snx", [128, NT, 128], bf16, kind="ExternalInput").ap()
    cmw = nc.dram_tensor("cmw", [128, NQC * DG], bf16, kind="ExternalInput").ap()
    out = nc.dram_tensor("out", [S, HID], bf16, kind="ExternalOutput").ap()
    _emit(nc, xT, wqT, wkvT, woT, csx, snx, cmw, out)
    nc.compile()
    return nc


def _get_compiled():
    global _compiled
    if _compiled is None:
        _compiled = _build()
    return _compiled


def _causal_masks():
    """cmw[k, ktl, q]: per diagonal-position wide mask over a 512-q chunk."""
    m = np.zeros((128, NQC, DG), np.float32)
    tri = np.triu(np.ones((128, 128), np.float32))  # 1 where k <= q
    for ktl in range(4):
        for qt in range(4):
            blk = m[:, ktl, qt * 128:(qt + 1) * 128]
            if qt > ktl:
                blk[:] = 1.0
            elif qt == ktl:
                blk[:] = tri
    return np.ascontiguousarray(
        m.reshape(128, NQC * DG).astype(ml_dtypes.bfloat16))


def kernel(x, cos, sin, wq, wk, wv, wo):
    nc = _get_compiled()
    x = np.asarray(x, np.float32)
    cos = np.asarray(cos, np.float32)
    sin = np.asarray(sin, np.float32)
    wq = np.asarray(wq, np.float32)
    wk = np.asarray(wk, np.float32)
    wv = np.asarray(wv, np.float32)
    wo = np.asarray(wo, np.float32)

    bf = ml_dtypes.bfloat16
    wkvT = np.ascontiguousarray(np.concatenate([wk, wv], 0).T.astype(bf))
    # cos duplicated halves; sin stored [sin | -sin]; blocked [p, st, d]
    csw = np.concatenate([cos, cos], 1).reshape(NT, 128, 128)
    snw = np.concatenate([sin, -sin], 1).reshape(NT, 128, 128)
    csx = np.ascontiguousarray(csw.transpose(1, 0, 2).astype(bf))
    snx = np.ascontiguousarray(snw.transpose(1, 0, 2).astype(bf))
    cmw = _causal_masks()
    # x blocked [p, st, t, s]: xT[p, st, t, s] = x[b].T[t*128+p, st*128+s]
    xTs = []
    for b in range(B):
        xt = x[b].T.astype(bf).reshape(HT, 128, NT, 128)
        xTs.append(np.ascontiguousarray(xt.transpose(1, 2, 0, 3)))
    wqTs = [np.ascontiguousarray(wq[g * DG:(g + 1) * DG].T.astype(bf))
            for g in range(GROUPS)]
    woTs = [np.ascontiguousarray(wo[:, g * DG:(g + 1) * DG].T.astype(bf))
            for g in range(GROUPS)]

    in_maps = []
    for c in range(NCORES):
        b, g = divmod(c, GROUPS)
        in_maps.append({
            "xT": xTs[b], "wqT": wqTs[g], "wkvT": wkvT, "woT": woTs[g],
            "csx": csx, "snx": snx, "cmw": cmw,
        })
    res = run_bass_kernel_spmd(nc, in_maps, list(range(NCORES)), trace=TRACE)
    LAST["res"] = res
    outs = [r["out"] for r in res.results]
    final = np.empty((B, S, HID), np.float32)
    for b in range(B):
        final[b] = (outs[GROUPS * b].astype(np.float32)
                    + outs[GROUPS * b + 1].astype(np.float32)
                    + outs[GROUPS * b + 2].astype(np.float32)
                    + outs[GROUPS * b + 3].astype(np.float32))
    return final


# revision 16
# speedup vs baseline: 1.3120x; 1.0045x over previous
"""Causal MQA self-attention (RoPE + RMS-norm on q/k) on 8 TRN2 NeuronCores.

Sharding: core c -> (batch b = c//4, head-group g = c%4 of 4 heads).
Each core computes, for its batch and its 4 heads:
  q/k/v projections -> RoPE -> RMS-norm -> causal attention -> partial
  output projection out_part = attn_out_g @ wo[:, g].T  (shape [S, HID]).
Host sums the 4 per-group partials of each batch (row-parallel matmul
unshard) and stacks the 2 batches.

v3 over the baseline:
  - weight DMAs split into chunks on the scalar HWDGE queue; x/cos/sin
    on the sync queue -> first matmul ~2us in (was ~34us).
  - x host-repacked into [p, st, t, s] blocks: 2KB contiguous runs.
  - RoPE without rotate-copies: sin is stored [sin | -sin] (host) and
    the rotate term is built by two half-width DVE muls that read the
    projection PSUM directly (no PSUM->SBUF staging for q, k at all).
  - k's RMS-norm is folded into the softmax exp as a per-k-partition
    scale AP (kinv); q's norm carries the 1/sqrt(D) factor.
  - the 5 per-stile transposes (4 q heads + k) land in one PSUM bank
    and drain with one wide scalar-engine copy each for q / k.
  - attention chunks split A1/A2 (head pairs) + deferred B half
    (transposes/out-proj/store), interleaved between stiles.
  - diagonal score matmuls narrowed to the unmasked span; output is
    stored bf16 as whole rows; gpsimd does nothing but stores.
"""

import ml_dtypes
import numpy as np

import concourse.bass as bass
import concourse.mybir as mybir
import concourse.tile as tile
from concourse import bacc
from concourse.bass_utils import run_bass_kernel_spmd
from concourse.masks import make_identity

# problem dims (hardcoded per contract)
B, S, HID, H, D = 2, 2048, 2048, 16, 128
NCORES = 8
GROUPS = 4              # head-groups = cores per batch
HG = H // GROUPS        # heads per core
DG = HG * D             # 512 projected q dims per core
NT = S // 128           # 16 sequence tiles
HT = HID // 128         # 16 hidden tiles
NQC = 4                 # q chunks of 512 columns
EPS = 1.1920928955078125e-07
ISD = 1.0 / float(np.sqrt(D))

f32 = mybir.dt.float32
bf16 = mybir.dt.bfloat16

TRACE = False           # test harness may flip this for NTFF profiling
LAST = {}               # last BassKernelResults, for the test harness
PH1_TILES = NT          # bisect knob
PH2_CHUNKS = NQC        # bisect knob

_compiled = None


def _bc(src, n):
    """[128, F] view -> [128, n, F] AP broadcasting along a step-0 mid dim."""
    return bass.AP(
        tensor=src.tensor,
        offset=src.offset,
        ap=[src.ap[0], [0, n], src.ap[1]],
    )


def _emit(nc, xT, wqT, wkvT, woT, csx, snx, cmw, out):
    add = mybir.AluOpType.add
    mult = mybir.AluOpType.mult
    Sqrt = mybir.ActivationFunctionType.Sqrt
    Exp = mybir.ActivationFunctionType.Exp

    with tile.TileContext(nc) as tc:
        with (
            tc.tile_pool(name="consts", bufs=1) as consts,
            tc.tile_pool(name="bigp", bufs=1) as bigp,
            tc.tile_pool(name="xsp", bufs=6) as xsp,
            tc.tile_pool(name="csp", bufs=3) as csp,
            tc.tile_pool(name="rsp", bufs=2) as rsp,
            tc.tile_pool(name="smp", bufs=4) as smp,
            tc.tile_pool(name="qnp", bufs=4) as qnp,
            tc.tile_pool(name="ptp", bufs=34) as ptp,
            tc.tile_pool(name="otp", bufs=2) as otp,
            tc.tile_pool(name="ocp", bufs=2) as ocp,
            tc.tile_pool(name="pPR", bufs=2, space="PSUM") as pPR,
            tc.tile_pool(name="pSC", bufs=3, space="PSUM") as pSC,
            tc.tile_pool(name="pOP", bufs=2, space="PSUM") as pOP,
        ):
            # ---- constants (tiny, engine-local) ----
            ident = consts.tile([128, 128], bf16)
            make_identity(nc, ident)
            eps_t = consts.tile([128, 1], f32)
            nc.vector.memset(eps_t, EPS)

            # ---- resident weights: split DMAs on scalar HWDGE queue ----
            wq_sb = bigp.tile([128, HT, DG], bf16, tag="wq")
            wqr = wqT.rearrange("(t p) d -> p t d", p=128)
            for c in range(4):
                nc.scalar.dma_start(wq_sb[:, 4 * c:4 * c + 4, :],
                                    wqr[:, 4 * c:4 * c + 4, :])
            wkv_sb = bigp.tile([128, HT, 2 * D], bf16, tag="wkv")
            wkvr = wkvT.rearrange("(t p) d -> p t d", p=128)
            for c in range(2):
                nc.scalar.dma_start(wkv_sb[:, 8 * c:8 * c + 8, :],
                                    wkvr[:, 8 * c:8 * c + 8, :])
            cmw_sb = consts.tile([128, NQC, DG], bf16)  # wide causal masks
            wo_sb = bigp.tile([128, HG, HID], bf16, tag="wo")
            wor = woT.rearrange("(h p) n -> p h n", p=128)

            qT_all = bigp.tile([128, HG, S], bf16, tag="qT")   # [d, h, s]
            kT_sb = bigp.tile([128, S], bf16, tag="kT")        # [d, s]
            vvb = bigp.tile([128, NT, 132], bf16, tag="vv")    # [s%128, s//128, d|ones]
            nc.vector.memset(vvb[:, :, 128:132], 1.0)
            kinv_all = bigp.tile([128, NT], f32, tag="kinv")   # per-k exp scales

            def emit_st_proj(st):
                # x blocked [p, st, t, s]: per-partition 2KB contiguous runs
                xs0 = xsp.tile([128, HT // 2, 128], bf16, tag="xs")
                nc.sync.dma_start(xs0, xT[:, st, 0:HT // 2, :])
                xs1 = xsp.tile([128, HT // 2, 128], bf16, tag="xs")
                nc.sync.dma_start(xs1, xT[:, st, HT // 2:HT, :])
                xhalves = (xs0, xs1)

                cs_t = csp.tile([128, 128], bf16, tag="cs")
                nc.sync.dma_start(cs_t, csx[:, st, :])
                sn_t = csp.tile([128, 128], bf16, tag="sn")   # [sin | -sin]
                nc.sync.dma_start(sn_t, snx[:, st, :])

                if st == 1:
                    nc.scalar.dma_start(
                        cmw_sb, cmw.rearrange("p (k q) -> p k q", k=NQC))
                elif st == 5:
                    # wo not needed until the first out-projection (B(0))
                    for c in range(2):
                        nc.scalar.dma_start(wo_sb[:, 2 * c:2 * c + 2, :],
                                            wor[:, 2 * c:2 * c + 2, :])

                qp = pPR.tile([128, DG], f32, tag="qp")
                for t in range(HT):
                    nc.tensor.matmul(
                        qp, lhsT=xhalves[t // 8][:, t % 8, :],
                        rhs=wq_sb[:, t, :], start=(t == 0), stop=(t == HT - 1),
                    )
                kvp = pPR.tile([128, 2 * D], f32, tag="kvp", bufs=1)
                for t in range(HT):
                    nc.tensor.matmul(
                        kvp, lhsT=xhalves[t // 8][:, t % 8, :],
                        rhs=wkv_sb[:, t, :], start=(t == 0), stop=(t == HT - 1),
                    )
                nc.vector.tensor_copy(vvb[:, st, 0:128], kvp[:, D:2 * D])
                return qp, kvp, cs_t, sn_t

            def emit_st_fin(st, staged):
                qp, kvp, cs_t, sn_t = staged
                # ---- RoPE + RMS-norm for 4 q heads, batched ----
                # rot = [q_hi * sin, q_lo * -sin] via half-width muls that
                # read the projection PSUM directly (fp32 DVE has no 2x
                # mode to lose); t1 = q * cos + rot.
                q3 = qp.rearrange("p (h d) -> p h d", h=HG)
                q4 = qp.rearrange("p (h t d) -> p h t d", h=HG, t=2)
                rot = rsp.tile([128, DG], f32, tag="rot")
                r4 = rot.rearrange("p (h t d) -> p h t d", h=HG, t=2)
                nc.vector.tensor_mul(r4[:, :, 0, :], q4[:, :, 1, :],
                                     _bc(sn_t[:, 0:64], HG))
                nc.vector.tensor_mul(r4[:, :, 1, :], q4[:, :, 0, :],
                                     _bc(sn_t[:, 64:128], HG))
                t1 = rsp.tile([128, DG], f32, tag="t1")
                t3 = t1.rearrange("p (h d) -> p h d", h=HG)
                nc.vector.tensor_mul(t3, q3, _bc(cs_t, HG))
                nc.vector.tensor_add(t1, t1, rot)          # t1 = roped q
                ms4 = smp.tile([128, HG], f32, tag="ms4")
                nc.vector.tensor_mul(rot, t1, t1)          # rot dead; reuse as q^2
                nc.vector.tensor_reduce(
                    ms4, rot.rearrange("p (h d) -> p h d", h=HG),
                    axis=mybir.AxisListType.X, op=add)
                srt4 = smp.tile([128, HG], f32, tag="srt4")
                nc.scalar.activation(out=srt4, in_=ms4, func=Sqrt,
                                     bias=eps_t[:, 0:1], scale=1.0 / D)
                nc.vector.reciprocal(srt4, srt4)
                # qn = t1 * ISD * srt4[h]  (ISD rides here; k-norm rides exp)
                qn_all = qnp.tile([128, DG], bf16, tag="qn")
                nc.vector.scalar_tensor_tensor(
                    qn_all.rearrange("p (h d) -> p h d", h=HG),
                    t1.rearrange("p (h d) -> p h d", h=HG),
                    ISD,
                    bass.AP(tensor=srt4.tensor, offset=srt4.offset,
                            ap=[srt4.ap[0], srt4.ap[1], [0, 128]]),
                    mult, mult)

                # ---- RoPE for k (single kv head); norm deferred to exp ----
                kk = kvp[:, 0:D]
                k2 = kk.rearrange("p (t d) -> p t d", t=2)
                krot = rsp.tile([128, 128], f32, tag="krot")
                kr2 = krot.rearrange("p (t d) -> p t d", t=2)
                nc.vector.tensor_mul(kr2[:, 0, :], k2[:, 1, :], sn_t[:, 0:64])
                nc.vector.tensor_mul(kr2[:, 1, :], k2[:, 0, :], sn_t[:, 64:128])
                kt1 = rsp.tile([128, 128], f32, tag="kt1")
                nc.vector.tensor_mul(kt1, kk, cs_t)
                nc.vector.tensor_add(kt1, kt1, krot)
                msk = smp.tile([128, 1], f32, tag="msk")
                nc.vector.tensor_mul(krot, kt1, kt1)       # krot dead; reuse as k^2
                nc.vector.tensor_reduce(msk, krot, axis=mybir.AxisListType.X, op=add)
                srtk = smp.tile([128, 1], f32, tag="srtk")
                nc.scalar.activation(out=srtk, in_=msk, func=Sqrt,
                                     bias=eps_t[:, 0:1], scale=1.0 / D)
                nc.vector.reciprocal(kinv_all[:, st:st + 1], srtk)
                kn = qnp.tile([128, 128], bf16, tag="kn")
                nc.vector.tensor_copy(kn, kt1)             # cast only

                # ---- 5 transposes into one PSUM bank, drained by 2 copies
                opt = pOP.tile([128, DG], f32, tag="op")
                opt_bf = opt.bitcast(bf16)                 # [128, 1024] bf16
                for h in range(HG):
                    nc.tensor.transpose(
                        opt_bf[:, h * 128:(h + 1) * 128],
                        qn_all[:, h * 128:(h + 1) * 128], ident)
                nc.tensor.transpose(opt_bf[:, 512:640], kn, ident)
                nc.vector.tensor_copy(
                    qT_all[:, :, st * 128:(st + 1) * 128],
                    opt_bf[:, 0:512].rearrange("p (h s) -> p h s", h=HG))
                nc.vector.tensor_copy(
                    kT_sb[:, st * 128:(st + 1) * 128], opt_bf[:, 512:640])

            def emit_qc_a(qc, heads):
                """scores -> exp (with per-k kinv scale) -> mask -> PV."""
                nkt = 4 * (qc + 1)
                pts = {}
                for h in heads:
                    for kt in range(nkt):
                        qoff = max(0, kt - 4 * qc) * 128
                        sp = pSC.tile([128, DG], f32, tag="sp")
                        nc.tensor.matmul(
                            sp[:, qoff:DG],
                            lhsT=kT_sb[:, kt * 128:(kt + 1) * 128],
                            rhs=qT_all[:, h, qc * DG + qoff:(qc + 1) * DG],
                            start=True, stop=True)
                        pt = ptp.tile([128, DG], bf16, tag="pt")
                        nc.scalar.activation(
                            out=pt[:, qoff:DG], in_=sp[:, qoff:DG],
                            func=Exp, scale=kinv_all[:, kt:kt + 1])
                        pts[(h, kt)] = pt
                    # causal masking of the diagonal k tiles (tri block)
                    for qtl in range(4):
                        kt = 4 * qc + qtl
                        sl = pts[(h, kt)][:, qtl * 128:(qtl + 1) * 128]
                        nc.vector.tensor_mul(
                            sl, sl, cmw_sb[:, qtl, qtl * 128:(qtl + 1) * 128])
                # probs @ [v | ones] per q tile, then normalize -> on
                for h in heads:
                    for qtl in range(4):
                        qt = 4 * qc + qtl
                        op = pOP.tile([128, DG], f32, tag="op")
                        for kt in range(qt + 1):
                            nc.tensor.matmul(
                                op[:, 0:129],
                                lhsT=pts[(h, kt)][:, qtl * 128:(qtl + 1) * 128],
                                rhs=vvb[:, kt, 0:129],
                                start=(kt == 0), stop=(kt == qt))
                        rc = smp.tile([128, 1], f32, tag="rc")
                        nc.vector.reciprocal(rc, op[:, 128:129])
                        on = qnp.tile([128, 128], bf16, tag="on", bufs=20)
                        nc.vector.tensor_scalar_mul(on, op[:, 0:128], rc)
                        ons[(h, qtl)] = on

            def emit_qc_b(qc):
                """transposes -> out projection -> bf16 store."""
                otile = otp.tile([128, HG, DG], bf16, tag="ot")  # [d, h, q]
                for qtl in range(4):
                    opt = pOP.tile([128, DG], f32, tag="op")
                    opt_bf = opt.bitcast(bf16)
                    for h in range(HG):
                        nc.tensor.transpose(
                            opt_bf[:, h * 128:(h + 1) * 128],
                            ons.pop((h, qtl)), ident)
                    nc.scalar.copy(
                        otile[:, :, qtl * 128:(qtl + 1) * 128],
                        opt_bf[:, 0:512].rearrange("p (h s) -> p h s", h=HG))
                for stl in range(4):
                    srow = (4 * qc + stl) * 128
                    ocw = ocp.tile([128, HID], bf16, tag="ocw")
                    for cc in range(4):
                        wop = pOP.tile([128, DG], f32, tag="op")
                        for h2 in range(HG):
                            nc.tensor.matmul(
                                wop,
                                lhsT=otile[:, h2, stl * 128:(stl + 1) * 128],
                                rhs=wo_sb[:, h2, cc * DG:(cc + 1) * DG],
                                start=(h2 == 0), stop=(h2 == HG - 1))
                        if cc % 2 == 0:
                            nc.vector.tensor_copy(ocw[:, cc * DG:(cc + 1) * DG], wop)
                        else:
                            nc.scalar.copy(ocw[:, cc * DG:(cc + 1) * DG], wop)
                    nc.sync.dma_start(out[srow:srow + 128, :], ocw)

            ons = {}
            # schedule: stile projections run one ahead of the rope/norm
            # finish (fills the DVE-chain latency with solid matmuls), and
            # chunk qc's attention is split into head pairs A1/A2 plus a
            # deferred back-half B, each slotted between stile finishes.
            staged = {}

            def after_fin(f):
                qc = f // 4
                if f % 4 == 3 and qc < PH2_CHUNKS:
                    emit_qc_a(qc, (0, 1))
                elif f % 4 == 0 and f > 0 and qc - 1 < PH2_CHUNKS:
                    emit_qc_a(qc - 1, (2, 3))
                elif f % 4 == 1 and f > 1 and qc - 1 < PH2_CHUNKS:
                    emit_qc_b(qc - 1)

            for st in range(PH1_TILES):
                staged[st] = emit_st_proj(st)
                if st >= 1:
                    emit_st_fin(st - 1, staged.pop(st - 1))
                    after_fin(st - 1)
            last = PH1_TILES - 1
            emit_st_fin(last, staged.pop(last))
            after_fin(last)
            if PH1_TILES == NT and PH2_CHUNKS == NQC:
                emit_qc_a(3, (2, 3))
                emit_qc_b(3)


def _build():
    nc = bacc.Bacc("TRN2", target_bir_lowering=False, debug=False,
                   num_devices=NCORES)
    # x blocked [p, st, t, s]; cos/sin blocked [p, st, d]
    xT = nc.dram_tensor("xT", [128, NT, HT, 128], bf16, kind="ExternalInput").ap()
    wqT = nc.dram_tensor("wqT", [HID, DG], bf16, kind="ExternalInput").ap()
    wkvT = nc.dram_tensor("wkvT", [HID, 2 * D], bf16, kind="ExternalInput").ap()
    woT = nc.dram_tensor("woT", [DG, HID], bf16, kind="ExternalInput").ap()
    csx = nc.dram_tensor("csx", [128, NT, 128], bf16, kind="ExternalInput").ap()
    snx = nc.dram_tensor("snx", [128, NT, 128], bf16, kind="ExternalInput").ap()
    cmw = nc.dram_tensor("cmw", [128, NQC * DG], bf16, kind="ExternalInput").ap()
    out = nc.dram_tensor("out", [S, HID], bf16, kind="ExternalOutput").ap()
    _emit(nc, xT, wqT, wkvT, woT, csx, snx, cmw, out)
    nc.compile()
    return nc


def _get_compiled():
    global _compiled
    if _compiled is None:
        _compiled = _build()
    return _compiled


def _causal_masks():
    """cmw[k, ktl, q]: per diagonal-position wide mask over a 512-q chunk."""
    m = np.zeros((128, NQC, DG), np.float32)
    tri = np.triu(np.ones((128, 128), np.float32))  # 1 where k <= q
    for ktl in range(4):
        for qt in range(4):
            blk = m[:, ktl, qt * 128:(qt + 1) * 128]
            if qt > ktl:
                blk[:] = 1.0
            elif qt == ktl:
                blk[:] = tri
    return np.ascontiguousarray(
        m.reshape(128, NQC * DG).astype(ml_dtypes.bfloat16))


def kernel(x, cos, sin, wq, wk, wv, wo):
    nc = _get_compiled()
    x = np.asarray(x, np.float32)
    cos = np.asarray(cos, np.float32)
    sin = np.asarray(sin, np.float32)
    wq = np.asarray(wq, np.float32)
    wk = np.asarray(wk, np.float32)
    wv = np.asarray(wv, np.float32)
    wo = np.asarray(wo, np.float32)

    bf = ml_dtypes.bfloat16
    wkvT = np.ascontiguousarray(np.concatenate([wk, wv], 0).T.astype(bf))
    # cos duplicated halves; sin stored [sin | -sin]; blocked [p, st, d]
    csw = np.concatenate([cos, cos], 1).reshape(NT, 128, 128)
    snw = np.concatenate([sin, -sin], 1).reshape(NT, 128, 128)
    csx = np.ascontiguousarray(csw.transpose(1, 0, 2).astype(bf))
    snx = np.ascontiguousarray(snw.transpose(1, 0, 2).astype(bf))
    cmw = _causal_masks()
    # x blocked [p, st, t, s]: xT[p, st, t, s] = x[b].T[t*128+p, st*128+s]
    xTs = []
    for b in range(B):
        xt = x[b].T.astype(bf).reshape(HT, 128, NT, 128)
        xTs.append(np.ascontiguousarray(xt.transpose(1, 2, 0, 3)))
    wqTs = [np.ascontiguousarray(wq[g * DG:(g + 1) * DG].T.astype(bf))
            for g in range(GROUPS)]
    woTs = [np.ascontiguousarray(wo[:, g * DG:(g + 1) * DG].T.astype(bf))
            for g in range(GROUPS)]

    in_maps = []
    for c in range(NCORES):
        b, g = divmod(c, GROUPS)
        in_maps.append({
            "xT": xTs[b], "wqT": wqTs[g], "wkvT": wkvT, "woT": woTs[g],
            "csx": csx, "snx": snx, "cmw": cmw,
        })
    res = run_bass_kernel_spmd(nc, in_maps, list(range(NCORES)), trace=TRACE)
    LAST["res"] = res
    outs = [r["out"] for r in res.results]
    final = np.empty((B, S, HID), np.float32)
    for b in range(B):
        final[b] = (outs[GROUPS * b].astype(np.float32)
                    + outs[GROUPS * b + 1].astype(np.float32)
                    + outs[GROUPS * b + 2].astype(np.float32)
                    + outs[GROUPS * b + 3].astype(np.float32))
    return final
